# revision 1
# baseline (speedup 1.0000x reference)
"""Trainium2 Bass kernel: paged int8-KV-cache GQA decode attention, 8-core SPMD.

Contract: kernel(**inputs) takes the FULL unsharded numpy inputs (as produced by
the reference setup_inputs) and returns the FULL [32, 4096] float32 output.

Strategy (pure data parallel over sequences, per the sharding hint):
  - 32 decode sequences are sorted by context length and dealt across the
    8 cores (one per length-octile slot), so every core owns 4 sequences and
    runs an identical, statically-shaped program.
  - Host staging is permutation/layout only: the int8-valued int32 KV cache is
    gathered per block_tables into per-core packed buffers (K transposed to
    [kvh, d, tokens], V natural [kvh, tokens, d]) and uploaded as int32.
    The new decode token is quantized and scattered exactly as the reference's
    store_kvcache does, before the gather.
  - On device, SWDGE DMAs cast int32 -> bf16 inline during the HBM->SBUF load
    (no separate dequant pass); k_scale * softmax_scale and v_scale are folded
    in as per-token vectors after the QK matmul / after exp respectively.
  - Per (slot, kv_head, 128-token tile):
      scores^T [128t, 4h] = matmul(lhsT=K^T tile [128d,128t], rhs=q^T [128d,4h])
      s1 = scores^T * ksb  (DVE; ksb = k_scale*SCALE, zeroed beyond ctx)
      e  = exp(s1) in bf16 (ACT)
      em = e * mask01, ev = e * v_scale_vec (DVE)
      Z  = two-stage PE reduction of em over tokens (partition axis)
      out += matmul(lhsT=ev [128t,4h], rhs=V tile [128t,128d]) accumulated in
      PSUM; at slot end out_h = pv / Z.
  Softmax skips max-subtraction (scores are O(20) at most; fp32 exp is safe).
"""

import os
import sys
import math
from contextlib import ExitStack

import numpy as np

sys.path.insert(0, "/opt/trn_rl_repo")

import ml_dtypes  # noqa: E402

import concourse.bass as bass  # noqa: E402
import concourse.mybir as mybir  # noqa: E402
import concourse.tile as tile  # noqa: E402
from concourse import bacc  # noqa: E402
from concourse.bass_utils import run_bass_kernel_spmd  # noqa: E402

BF16 = ml_dtypes.bfloat16

B = 32
NUM_HEADS = 32
KVH = 8
D = 128
REP = NUM_HEADS // KVH  # 4
BLOCK_SIZE = 256
T = 4096
P = 128
SCALE = 1.0 / float(np.sqrt(D))
NCORES = 8
SLOTS = 4


# ---------------------------------------------------------------------------
# host-side planning + packing
# ---------------------------------------------------------------------------

def _plan(context_lens):
    """Assign sequences to (core, slot); slot tile counts = octile maxima.

    Each slot's K/V loads are split into up to 3 pieces; pieces past a
    sequence's length are skipped per-core via predicated DMAs.  Returns
    (assign, ns, pieces) where pieces[s] is the list of piece boundaries.
    """
    order = np.argsort(-context_lens, kind="stable")  # descending
    ns = []
    assign = np.zeros((NCORES, SLOTS), dtype=np.int64)
    for s in range(SLOTS):
        octile = order[8 * s: 8 * s + 8]
        ns.append(int(math.ceil(int(context_lens[octile[0]]) / P)))
        # alternate direction per slot to roughly balance true work
        ranks = octile if s % 2 == 0 else octile[::-1]
        for c in range(NCORES):
            assign[c, s] = ranks[c]

    # NOTE: runtime-predicated (cond=) piece skipping was tried to trim the
    # ~12% slot padding, but SWDGE cond-DMAs produced wrong data on HW even
    # with always-true flags, so loads are unconditional.
    pieces = [[0, n] for n in ns]
    return assign, ns, pieces


def _quantize(x):
    absmax = np.abs(x).max(axis=-1)
    scale = np.where(absmax > 0.0, absmax / 127.0, 1.0).astype(np.float32)
    xq = np.clip(np.round(x / scale[..., None]), -127.0, 127.0).astype(np.int32)
    return xq, scale


def _pack_inputs(inputs, assign, ns, pieces):
    q = inputs["q"].reshape(B, NUM_HEADS, D).astype(np.float32)
    k = inputs["k"].reshape(B, KVH, D).astype(np.float32)
    v = inputs["v"].reshape(B, KVH, D).astype(np.float32)
    kc = np.ascontiguousarray(inputs["k_cache_q"].reshape(-1, KVH, D))
    vc = np.ascontiguousarray(inputs["v_cache_q"].reshape(-1, KVH, D))
    ks = np.ascontiguousarray(inputs["k_scale"].reshape(-1, KVH)).astype(np.float32)
    vs = np.ascontiguousarray(inputs["v_scale"].reshape(-1, KVH)).astype(np.float32)
    bt = inputs["block_tables"]
    ctx = inputs["context_lens"]
    sm = inputs["slot_mapping"]

    # store_kvcache_int8: quantize the new token and scatter into the cache
    kq, ksn = _quantize(k)
    vq, vsn = _quantize(v)
    kc = kc.copy(); vc = vc.copy(); ks = ks.copy(); vs = vs.copy()
    kc[sm] = kq; vc[sm] = vq; ks[sm] = ksn; vs[sm] = vsn

    NTT = sum(ns)           # token tiles per core
    NT = NTT * P            # tokens per core
    offs = np.concatenate([[0], np.cumsum(ns)])

    in_maps = []
    for c in range(NCORES):
        kt_c = np.zeros((KVH, D, NT), dtype=np.int32)
        # V pre-tiled [kvh, partition, tile, d] so each partition's slot data
        # is one contiguous run for the DMA (avoids 256B-packet spray)
        vp_c = np.zeros((KVH, P, NTT, D), dtype=np.int32)
        ksb_c = np.zeros((P, NTT * KVH), dtype=np.float32)
        vsb_c = np.zeros((P, NTT * KVH), dtype=BF16)
        m01_c = np.zeros((P, NTT * KVH), dtype=BF16)
        qt_c = np.zeros((P, SLOTS * 32), dtype=np.float32)
        pf_c = np.zeros((1, 8), dtype=np.int32)
        for s in range(SLOTS):
            b = int(assign[c, s])
            n = ns[s]
            nt = n * P
            o = int(offs[s])
            act = int(math.ceil(int(inputs["context_lens"][b]) / P))
            for pi, st in enumerate(pieces[s][1:-1]):
                pf_c[0, (s - 1) * 2 + pi] = 1 if act > st else 0
            flat = (bt[b][:, None] * BLOCK_SIZE
                    + np.arange(BLOCK_SIZE, dtype=np.int64)[None, :]).reshape(-1)[:nt]
            kg = kc[flat]                      # [nt, KVH, D] int32
            vg = vc[flat]
            kt_c[:, :, o * P: o * P + nt] = kg.transpose(1, 2, 0)
            # [nt, KVH, D] -> [n, P, KVH, D] -> [KVH, P, n, D]
            vp_c[:, :, o: o + n, :] = vg.reshape(n, P, KVH, D).transpose(2, 1, 0, 3)
            valid = (np.arange(nt) < int(ctx[b]))
            ksg = (ks[flat] * SCALE) * valid[:, None]        # [nt, KVH]
            vsg = vs[flat] * valid[:, None]
            # [nt, KVH] -> [P, n*KVH]  (head-broadcast happens on device)
            def sprd(a, dt):
                return a.reshape(n, P, KVH).transpose(1, 0, 2).reshape(
                    P, n * KVH).astype(dt)
            ksb_c[:, o * KVH: (o + n) * KVH] = sprd(ksg, np.float32)
            vsb_c[:, o * KVH: (o + n) * KVH] = sprd(vsg, BF16)
            m01_c[:, o * KVH: (o + n) * KVH] = sprd(
                valid[:, None] * np.ones((1, KVH), np.float32), BF16)
            qt_c[:, s * 32: (s + 1) * 32] = q[b].transpose(1, 0)  # [D, 32]
        sel = np.tile(np.eye(4, dtype=np.float32), (32, 1))       # [128, 4]
        in_maps.append(dict(kt=kt_c, vp=vp_c, ksb=ksb_c, vsb=vsb_c,
                            m01=m01_c, qt=qt_c, sel=sel, pf=pf_c))
    return in_maps


# ---------------------------------------------------------------------------
# device program
# ---------------------------------------------------------------------------

def _kvh_body(nc, s, n, j, ktc, vtc, qt, sel, ones, ksb_s, vsb_s, m01_s,
              ps_qk, ps_pt, work, zts, pvts):
    """QK -> scale -> exp -> mask -> Z -> PV for one (slot, kv head)."""
    f32 = mybir.dt.float32
    bf16 = mybir.dt.bfloat16
    EXP = mybir.ActivationFunctionType.Exp

    qk = ps_qk.tile([P, n, 4], f32, tag="qk")
    qcol = s * 32 + 4 * j
    for i in range(n):
        nc.tensor.matmul(
            qk[:, i, :],
            lhsT=ktc[:, i, :],
            rhs=qt[:, qcol: qcol + 4],
            start=True, stop=True, skip_group_check=True)

    s1 = work.tile([P, n, 4], f32, tag="s1")
    nc.vector.tensor_mul(
        s1, qk, ksb_s[:, :, j: j + 1].to_broadcast([P, n, 4]))
    e = work.tile([P, n, 4], bf16, tag="e")
    nc.scalar.activation(e, s1, EXP)
    em = work.tile([P, n, 4], bf16, tag="em")
    nc.vector.tensor_mul(
        em, e, m01_s[:, :, j: j + 1].to_broadcast([P, n, 4]))
    ev = work.tile([P, n, 4], bf16, tag="ev")
    nc.vector.tensor_mul(
        ev, e, vsb_s[:, :, j: j + 1].to_broadcast([P, n, 4]))

    # Z: per-(tile, head) partial sums, then fold tiles via selector matmul
    pt = ps_pt.tile([P, 1], f32, tag="pt")
    nc.tensor.matmul(pt[0: n * 4, :], lhsT=em, rhs=ones,
                     start=True, stop=True)
    pts = work.tile([P, 1], f32, tag="pts")
    nc.vector.tensor_copy(pts[0: n * 4, :], pt[0: n * 4, :])
    zt = zts[j // 4]
    bp = 32 * (j % 4)
    nc.tensor.matmul(zt[bp: bp + 4, :], lhsT=sel[0: n * 4, :],
                     rhs=pts[0: n * 4, :], start=True, stop=True,
                     tile_position=(0, bp))

    # PV accumulate over token tiles
    pvt = pvts[j // 4]
    for i in range(n):
        nc.tensor.matmul(
            pvt[bp: bp + 4, :],
            lhsT=ev[:, i, :],
            rhs=vtc[:, i, :],
            start=(i == 0), stop=(i == n - 1),
            tile_position=(0, bp))


def _build_program(ns, pieces, no_cond=False, no_memset=False):
    NTT = sum(ns)
    NT = NTT * P
    offs = [0]
    for n in ns:
        offs.append(offs[-1] + n)
    f32 = mybir.dt.float32
    bf16 = mybir.dt.bfloat16
    i32 = mybir.dt.int32
    EXP = mybir.ActivationFunctionType.Exp

    nc = bacc.Bacc("TRN2", target_bir_lowering=False, debug=False,
                   num_devices=NCORES)

    kt_d = nc.dram_tensor("kt", [KVH, D, NT], i32, kind="ExternalInput").ap()
    vp_d = nc.dram_tensor("vp", [KVH, P, NTT, D], i32, kind="ExternalInput").ap()
    ksb_d = nc.dram_tensor("ksb", [P, NTT * KVH], f32, kind="ExternalInput").ap()
    vsb_d = nc.dram_tensor("vsb", [P, NTT * KVH], bf16, kind="ExternalInput").ap()
    m01_d = nc.dram_tensor("m01", [P, NTT * KVH], bf16, kind="ExternalInput").ap()
    qt_d = nc.dram_tensor("qt", [P, SLOTS * 32], f32, kind="ExternalInput").ap()
    sel_d = nc.dram_tensor("sel", [P, 4], f32, kind="ExternalInput").ap()
    pf_d = nc.dram_tensor("pf", [1, 8], mybir.dt.int32,
                          kind="ExternalInput").ap()
    out_d = nc.dram_tensor("out", [SLOTS, 2, P, D], f32,
                           kind="ExternalOutput").ap()

    with tile.TileContext(nc) as tc, ExitStack() as ctx:
        const = ctx.enter_context(tc.tile_pool(name="const", bufs=1))
        kt_pool = ctx.enter_context(tc.tile_pool(name="ktp", bufs=2))
        v_pool = ctx.enter_context(tc.tile_pool(name="vpp", bufs=2))
        sc_pool = ctx.enter_context(tc.tile_pool(name="scp", bufs=2))
        work = ctx.enter_context(tc.tile_pool(name="wrk", bufs=3))
        ps_qk = ctx.enter_context(tc.tile_pool(name="psqk", bufs=2, space="PSUM"))
        ps_pt = ctx.enter_context(tc.tile_pool(name="pspt", bufs=2, space="PSUM"))
        ps_z = ctx.enter_context(tc.tile_pool(name="psz", bufs=1, space="PSUM"))
        ps_pv = ctx.enter_context(tc.tile_pool(name="pspv", bufs=1, space="PSUM"))

        qt_f = const.tile([P, SLOTS * 32], f32)
        nc.sync.dma_start(qt_f, qt_d)
        qt = const.tile([P, SLOTS * 32], bf16)
        nc.vector.tensor_copy(qt, qt_f)
        sel = const.tile([P, 4], f32)
        nc.sync.dma_start(sel, sel_d)
        ones = const.tile([P, 1], bf16)
        nc.vector.memset(ones, 1.0)
        conds = {}
        if any(len(p) > 2 for p in pieces):
            pf_sb = const.tile([1, 8], mybir.dt.int32)
            nc.sync.dma_start(pf_sb, pf_d)
            for s in range(1, SLOTS):
                for pi in range(len(pieces[s]) - 2):
                    reg = nc.alloc_registers(f"pf_{s}_{pi}",
                                             engines=[mybir.EngineType.Pool])
                    nc.regs_load(reg, pf_sb[0:1, (s - 1) * 2 + pi:
                                            (s - 1) * 2 + pi + 1])
                    conds[(s, pi)] = nc.snap(reg, donate=True,
                                             min_val=0, max_val=1)

        for s in range(SLOTS):
            n = ns[s]
            o = offs[s]
            ksb_s = sc_pool.tile([P, n, KVH], f32, tag="ksb")
            nc.sync.dma_start(ksb_s, ksb_d[:, o * KVH: (o + n) * KVH])
            vsb_s = sc_pool.tile([P, n, KVH], bf16, tag="vsb")
            nc.sync.dma_start(vsb_s, vsb_d[:, o * KVH: (o + n) * KVH])
            m01_s = sc_pool.tile([P, n, KVH], bf16, tag="m01")
            nc.sync.dma_start(m01_s, m01_d[:, o * KVH: (o + n) * KVH])

            pv0 = ps_pv.tile([P, D], f32, tag="pv0")
            pv1 = ps_pv.tile([P, D], f32, tag="pv1")
            z0 = ps_z.tile([P, 1], f32, tag="z0")
            z1 = ps_z.tile([P, 1], f32, tag="z1")
            nc.vector.memset(pv0, 0.0)
            nc.vector.memset(pv1, 0.0)
            nc.vector.memset(z0, 1.0)
            nc.vector.memset(z1, 1.0)

            for jq in range(KVH // 4):
                ktc = kt_pool.tile([P, 4, n, P], bf16, tag="kt")
                vtc = v_pool.tile([P, 4, n, D], bf16, tag="vt")
                nc.gpsimd.dma_start(
                    ktc,
                    kt_d[4 * jq: 4 * jq + 4, :,
                         o * P: (o + n) * P].rearrange("j d t -> d j t"))
                nc.gpsimd.dma_start(
                    vtc,
                    vp_d[4 * jq: 4 * jq + 4, :, o: o + n,
                         :].rearrange("j p i d -> p j i d"))
                for j2 in range(4):
                    _kvh_body(nc, s, n, 4 * jq + j2,
                              ktc[:, j2], vtc[:, j2],
                              qt, sel, ones, ksb_s, vsb_s, m01_s,
                              ps_qk, ps_pt, work,
                              (z0, z1), (pv0, pv1))

            rz0 = work.tile([P, 1], f32, tag="rz0")
            nc.vector.reciprocal(rz0, z0)
            rz1 = work.tile([P, 1], f32, tag="rz1")
            nc.vector.reciprocal(rz1, z1)
            o0 = work.tile([P, D], f32, tag="o0")
            nc.vector.tensor_scalar_mul(o0, pv0, rz0)
            o1 = work.tile([P, D], f32, tag="o1")
            nc.vector.tensor_scalar_mul(o1, pv1, rz1)
            nc.sync.dma_start(out_d[s, 0], o0)
            nc.sync.dma_start(out_d[s, 1], o1)

    nc.compile()
    return nc


_PROGRAM_CACHE = {}


def _get_program(ns, pieces):
    key = (tuple(ns), tuple(tuple(p) for p in pieces))
    if key not in _PROGRAM_CACHE:
        _PROGRAM_CACHE[key] = _build_program(ns, pieces)
    return _PROGRAM_CACHE[key]


# ---------------------------------------------------------------------------
# entry point
# ---------------------------------------------------------------------------

def kernel(q, k, v, k_cache_q, v_cache_q, k_scale, v_scale,
           block_tables, context_lens, slot_mapping, _trace=False):
    inputs = dict(q=np.asarray(q), k=np.asarray(k), v=np.asarray(v),
                  k_cache_q=np.asarray(k_cache_q),
                  v_cache_q=np.asarray(v_cache_q),
                  k_scale=np.asarray(k_scale), v_scale=np.asarray(v_scale),
                  block_tables=np.asarray(block_tables),
                  context_lens=np.asarray(context_lens),
                  slot_mapping=np.asarray(slot_mapping))
    assign, ns, pieces = _plan(inputs["context_lens"])
    in_maps = _pack_inputs(inputs, assign, ns, pieces)
    nc = _get_program(ns, pieces)
    res = run_bass_kernel_spmd(nc, in_maps, core_ids=list(range(NCORES)),
                               trace=_trace)

    out = np.zeros((B, NUM_HEADS, D), dtype=np.float32)
    for c in range(NCORES):
        oc = res.results[c]["out"]  # [SLOTS, 2, P, D]
        for s in range(SLOTS):
            b = int(assign[c, s])
            for j in range(KVH):
                bp = 32 * (j % 4)
                out[b, 4 * j: 4 * j + 4] = oc[s, j // 4, bp: bp + 4, :]
    out = out.reshape(B, NUM_HEADS * D)
    if _trace:
        return out, res
    return out



# revision 3
# speedup vs baseline: 1.9879x; 1.9879x over previous
"""Trainium2 Bass kernel: paged int8-KV-cache GQA decode attention, 8-core SPMD.

Contract: kernel(**inputs) takes the FULL unsharded numpy inputs (as produced by
the reference setup_inputs) and returns the FULL [32, 4096] float32 output.

Strategy (pure data parallel over sequences, per the sharding hint):
  - 32 decode sequences are sorted by context length and dealt across the
    8 cores (one per length-octile slot), so every core owns 4 sequences and
    runs an identical, statically-shaped program.
  - Host staging is permutation/layout only: the int8-valued int32 KV cache is
    gathered per block_tables into per-core packed **int8** buffers (K
    transposed to [kvh, d, tokens], V natural [kvh, tokens, d]).  The new
    decode token is quantized and scattered exactly as the reference's
    store_kvcache does, before the gather.
  - On device, SWDGE DMAs cast int8 -> bf16 inline during the HBM->SBUF load
    (4x less HBM traffic than the int32 container); k_scale * softmax_scale
    and v_scale are folded in as per-token vectors after the QK matmul / after
    exp respectively.
  - Per (slot, group of 4 kv heads):
      scores [128t, 4kvh, n, 4h] = per-tile matmuls(lhsT=K^T tile, rhs=q^T)
      s1 = scores * ksb  (DVE; ksb = k_scale*SCALE, zeroed beyond ctx)
      e  = exp(s1) in bf16 (ACT), one op per 4-kvh group
      ev = e * v_scale_vec (DVE; v_scale zeroed beyond ctx)
      Z  = matmul(lhsT=e, rhs=ones) per kvh -> fold to [1, 32]; the pad
           region contributes exp(0)=1 per token, corrected by subtracting a
           host-computed pad count (no mask multiply needed)
      PV = matmul(lhsT=V tile [128t,128d], rhs=ev [128t,4]) accumulated in
           PSUM as out^T [128d, 4h]; at slot end out = pv * (1/Z) with 1/Z
           broadcast across partitions via a rank-1 matmul.
  Softmax skips max-subtraction (scores are O(20) at most; fp32 exp is safe).
"""

import os
import sys
import math
from contextlib import ExitStack

import numpy as np

sys.path.insert(0, "/opt/trn_rl_repo")

import ml_dtypes  # noqa: E402

import concourse.bass as bass  # noqa: E402
import concourse.mybir as mybir  # noqa: E402
import concourse.tile as tile  # noqa: E402
from concourse import bacc  # noqa: E402
from concourse.bass_utils import run_bass_kernel_spmd  # noqa: E402

BF16 = ml_dtypes.bfloat16

B = 32
NUM_HEADS = 32
KVH = 8
D = 128
REP = NUM_HEADS // KVH  # 4
BLOCK_SIZE = 256
T = 4096
P = 128
SCALE = 1.0 / float(np.sqrt(D))
NCORES = 8
SLOTS = 4


# ---------------------------------------------------------------------------
# host-side planning + packing
# ---------------------------------------------------------------------------

def _plan(context_lens):
    """Assign sequences to (core, slot); slot tile counts = octile maxima."""
    order = np.argsort(-context_lens, kind="stable")  # descending
    ns = []
    assign = np.zeros((NCORES, SLOTS), dtype=np.int64)
    for s in range(SLOTS):
        octile = order[8 * s: 8 * s + 8]
        ns.append(int(math.ceil(int(context_lens[octile[0]]) / P)))
        # alternate direction per slot to roughly balance true work
        ranks = octile if s % 2 == 0 else octile[::-1]
        for c in range(NCORES):
            assign[c, s] = ranks[c]
    return assign, ns


def _quantize(x):
    absmax = np.abs(x).max(axis=-1)
    scale = np.where(absmax > 0.0, absmax / 127.0, 1.0).astype(np.float32)
    xq = np.clip(np.round(x / scale[..., None]), -127.0, 127.0).astype(np.int32)
    return xq, scale


def _pack_inputs(inputs, assign, ns):
    q = inputs["q"].reshape(B, NUM_HEADS, D).astype(np.float32)
    k = inputs["k"].reshape(B, KVH, D).astype(np.float32)
    v = inputs["v"].reshape(B, KVH, D).astype(np.float32)
    kc = np.ascontiguousarray(inputs["k_cache_q"].reshape(-1, KVH, D))
    vc = np.ascontiguousarray(inputs["v_cache_q"].reshape(-1, KVH, D))
    ks = np.ascontiguousarray(inputs["k_scale"].reshape(-1, KVH)).astype(np.float32)
    vs = np.ascontiguousarray(inputs["v_scale"].reshape(-1, KVH)).astype(np.float32)
    bt = inputs["block_tables"]
    ctx = inputs["context_lens"]
    sm = inputs["slot_mapping"]

    # store_kvcache_int8: quantize the new token and scatter into the cache
    kq, ksn = _quantize(k)
    vq, vsn = _quantize(v)
    kc = kc.copy(); vc = vc.copy(); ks = ks.copy(); vs = vs.copy()
    kc[sm] = kq; vc[sm] = vq; ks[sm] = ksn; vs[sm] = vsn

    NTT = sum(ns)           # token tiles per core
    NT = NTT * P            # tokens per core
    offs = np.concatenate([[0], np.cumsum(ns)])

    in_maps = []
    for c in range(NCORES):
        kt_c = np.zeros((KVH, D, NT), dtype=np.int8)
        # V pre-tiled [kvh, partition, tile, d] so each partition's slot data
        # is one contiguous run for the DMA
        vp_c = np.zeros((KVH, P, NTT, D), dtype=np.int8)
        # scales per slot, kvh-major then tile: [P, kvh, tiles]
        ksb_c = np.zeros((P, KVH * NTT), dtype=np.float32)
        vsb_c = np.zeros((P, KVH * NTT), dtype=BF16)
        qt_c = np.zeros((P, SLOTS * 32), dtype=np.float32)
        padc_c = np.zeros((1, SLOTS), dtype=np.float32)
        for s in range(SLOTS):
            b = int(assign[c, s])
            n = ns[s]
            nt = n * P
            o = int(offs[s])
            padc_c[0, s] = float(nt - int(ctx[b]))
            flat = (bt[b][:, None] * BLOCK_SIZE
                    + np.arange(BLOCK_SIZE, dtype=np.int64)[None, :]).reshape(-1)[:nt]
            kg = kc[flat]                      # [nt, KVH, D] int32 (int8 vals)
            vg = vc[flat]
            kt_c[:, :, o * P: o * P + nt] = kg.transpose(1, 2, 0)
            # [nt, KVH, D] -> [n, P, KVH, D] -> [KVH, P, n, D]
            vp_c[:, :, o: o + n, :] = vg.reshape(n, P, KVH, D).transpose(2, 1, 0, 3)
            valid = (np.arange(nt) < int(ctx[b]))
            ksg = (ks[flat] * SCALE) * valid[:, None]        # [nt, KVH]
            vsg = vs[flat] * valid[:, None]
            # [nt, KVH] -> [P, KVH * n]  (kvh-major; head-broadcast on device)
            def sprd(a, dt):
                return a.reshape(n, P, KVH).transpose(1, 2, 0).reshape(
                    P, KVH * n).astype(dt)
            ksb_c[:, o * KVH: (o + n) * KVH] = sprd(ksg, np.float32)
            vsb_c[:, o * KVH: (o + n) * KVH] = sprd(vsg, BF16)
            qt_c[:, s * 32: (s + 1) * 32] = q[b].transpose(1, 0)  # [D, 32]
        sel = np.tile(np.eye(4, dtype=np.float32), (32, 1))       # [128, 4]
        in_maps.append(dict(kt=kt_c, vp=vp_c, ksb=ksb_c, vsb=vsb_c,
                            qt=qt_c, sel=sel, padc=padc_c))
    return in_maps


# ---------------------------------------------------------------------------
# device program
# ---------------------------------------------------------------------------

def _build_program(ns):
    NTT = sum(ns)
    NT = NTT * P
    offs = [0]
    for n in ns:
        offs.append(offs[-1] + n)
    f32 = mybir.dt.float32
    bf16 = mybir.dt.bfloat16
    i8 = mybir.dt.int8
    EXP = mybir.ActivationFunctionType.Exp

    nc = bacc.Bacc("TRN2", target_bir_lowering=False, debug=False,
                   num_devices=NCORES)

    kt_d = nc.dram_tensor("kt", [KVH, D, NT], i8, kind="ExternalInput").ap()
    vp_d = nc.dram_tensor("vp", [KVH, P, NTT, D], i8, kind="ExternalInput").ap()
    ksb_d = nc.dram_tensor("ksb", [P, KVH * NTT], f32, kind="ExternalInput").ap()
    vsb_d = nc.dram_tensor("vsb", [P, KVH * NTT], bf16, kind="ExternalInput").ap()
    qt_d = nc.dram_tensor("qt", [P, SLOTS * 32], f32, kind="ExternalInput").ap()
    sel_d = nc.dram_tensor("sel", [P, 4], f32, kind="ExternalInput").ap()
    padc_d = nc.dram_tensor("padc", [1, SLOTS], f32, kind="ExternalInput").ap()
    out_d = nc.dram_tensor("out", [SLOTS, P, 32], f32,
                           kind="ExternalOutput").ap()

    with tile.TileContext(nc) as tc, ExitStack() as ctx:
        const = ctx.enter_context(tc.tile_pool(name="const", bufs=1))
        kt_pool = ctx.enter_context(tc.tile_pool(name="ktp", bufs=2))
        v_pool = ctx.enter_context(tc.tile_pool(name="vpp", bufs=2))
        sc_pool = ctx.enter_context(tc.tile_pool(name="scp", bufs=2))
        work = ctx.enter_context(tc.tile_pool(name="wrk", bufs=3))
        tail = ctx.enter_context(tc.tile_pool(name="tl", bufs=2))
        ps_qk = ctx.enter_context(tc.tile_pool(name="psqk", bufs=2, space="PSUM"))
        ps_pt = ctx.enter_context(tc.tile_pool(name="pspt", bufs=2, space="PSUM"))
        ps_z = ctx.enter_context(tc.tile_pool(name="psz", bufs=1, space="PSUM"))
        ps_pv = ctx.enter_context(tc.tile_pool(name="pspv", bufs=2, space="PSUM"))
        ps_zb = ctx.enter_context(tc.tile_pool(name="pszb", bufs=1, space="PSUM"))

        qt_f = const.tile([P, SLOTS * 32], f32)
        nc.sync.dma_start(qt_f, qt_d)
        qt = const.tile([P, SLOTS * 32], bf16)
        nc.vector.tensor_copy(qt, qt_f)
        sel = const.tile([P, 4], f32)
        nc.sync.dma_start(sel, sel_d)
        padc = const.tile([1, SLOTS], f32)
        nc.sync.dma_start(padc, padc_d)
        ones = const.tile([P, 1], bf16)
        nc.vector.memset(ones, 1.0)
        ones1 = const.tile([1, P], f32)
        nc.vector.memset(ones1, 1.0)

        for s in range(SLOTS):
            n = ns[s]
            o = offs[s]
            ksb_s = sc_pool.tile([P, KVH, n, 1], f32, tag="ksb")
            nc.sync.dma_start(ksb_s, ksb_d[:, o * KVH: (o + n) * KVH])
            vsb_s = sc_pool.tile([P, KVH, n, 1], bf16, tag="vsb")
            nc.sync.dma_start(vsb_s, vsb_d[:, o * KVH: (o + n) * KVH])

            pv = ps_pv.tile([P, 32], f32, tag="pv")
            pt = ps_pt.tile([P, KVH], f32, tag="pt")
            z_all = ps_z.tile([1, 32], f32, tag="z")

            for jq in range(KVH // 4):
                ktc = kt_pool.tile([P, 4, n, P], bf16, tag="kt")
                vtc = v_pool.tile([P, 4, n, D], bf16, tag="vt")
                nc.gpsimd.dma_start(
                    ktc,
                    kt_d[4 * jq: 4 * jq + 4, :,
                         o * P: (o + n) * P].rearrange("j d t -> d j t"))
                nc.gpsimd.dma_start(
                    vtc,
                    vp_d[4 * jq: 4 * jq + 4, :, o: o + n,
                         :].rearrange("j p i d -> p j i d"))

                # QK for the 4 kv heads of this group
                qk = ps_qk.tile([P, 4, n, 4], f32, tag="qk")
                for j2 in range(4):
                    qcol = s * 32 + 4 * (4 * jq + j2)
                    for i in range(n):
                        nc.tensor.matmul(
                            qk[:, j2, i, :],
                            lhsT=ktc[:, j2, i, :],
                            rhs=qt[:, qcol: qcol + 4],
                            start=True, stop=True, skip_group_check=True)

                # softmax pieces, batched over the 4-kvh group
                s1 = work.tile([P, 4, n, 4], f32, tag="s1")
                nc.vector.tensor_mul(
                    s1, qk,
                    ksb_s[:, 4 * jq: 4 * jq + 4].to_broadcast([P, 4, n, 4]))
                e = work.tile([P, 4, n, 4], bf16, tag="e")
                nc.scalar.activation(e, s1, EXP)
                ev = work.tile([P, 4, n, 4], bf16, tag="ev")
                nc.vector.tensor_mul(
                    ev, e,
                    vsb_s[:, 4 * jq: 4 * jq + 4].to_broadcast([P, 4, n, 4]))

                for j2 in range(4):
                    j = 4 * jq + j2
                    # Z partials: per-(tile, head) column sums of e
                    nc.tensor.matmul(
                        pt[0: n * 4, j: j + 1],
                        lhsT=e[:, j2], rhs=ones,
                        start=True, stop=True, skip_group_check=True)
                    # PV accumulate over token tiles: out^T [128d, 4h]
                    cc = 4 * j
                    for i in range(n):
                        nc.tensor.matmul(
                            pv[:, cc: cc + 4],
                            lhsT=vtc[:, j2, i, :],
                            rhs=ev[:, j2, i, :],
                            start=(i == 0), stop=(i == n - 1),
                            skip_group_check=True)

            # fold Z partials -> [1, 32], correct for exp(0)=1 pad terms
            pts = tail.tile([P, KVH], f32, tag="pts")
            nc.vector.tensor_copy(pts[0: n * 4, :], pt[0: n * 4, :])
            for j in range(KVH):
                nc.tensor.matmul(
                    z_all[0:1, 4 * j: 4 * j + 4],
                    lhsT=pts[0: n * 4, j: j + 1],
                    rhs=sel[0: n * 4, :],
                    start=True, stop=True, skip_group_check=True)
            zs = tail.tile([1, 32], f32, tag="zs")
            nc.vector.tensor_scalar_sub(zs, z_all, padc[0:1, s: s + 1])
            rz = tail.tile([1, 32], f32, tag="rz")
            nc.vector.reciprocal(rz, zs)
            # broadcast 1/Z across partitions with a rank-1 matmul
            zb = ps_zb.tile([P, 32], f32, tag="zb")
            nc.tensor.matmul(zb, lhsT=ones1, rhs=rz, start=True, stop=True)
            zbs = tail.tile([P, 32], f32, tag="zbs")
            nc.vector.tensor_copy(zbs, zb)
            o_sb = tail.tile([P, 32], f32, tag="o")
            nc.vector.tensor_mul(o_sb, pv, zbs)
            nc.sync.dma_start(out_d[s], o_sb)

    nc.compile()
    return nc


_PROGRAM_CACHE = {}


def _get_program(ns):
    key = tuple(ns)
    if key not in _PROGRAM_CACHE:
        _PROGRAM_CACHE[key] = _build_program(ns)
    return _PROGRAM_CACHE[key]


# ---------------------------------------------------------------------------
# entry point
# ---------------------------------------------------------------------------

def kernel(q, k, v, k_cache_q, v_cache_q, k_scale, v_scale,
           block_tables, context_lens, slot_mapping, _trace=False):
    inputs = dict(q=np.asarray(q), k=np.asarray(k), v=np.asarray(v),
                  k_cache_q=np.asarray(k_cache_q),
                  v_cache_q=np.asarray(v_cache_q),
                  k_scale=np.asarray(k_scale), v_scale=np.asarray(v_scale),
                  block_tables=np.asarray(block_tables),
                  context_lens=np.asarray(context_lens),
                  slot_mapping=np.asarray(slot_mapping))
    assign, ns = _plan(inputs["context_lens"])
    in_maps = _pack_inputs(inputs, assign, ns)
    nc = _get_program(ns)
    res = run_bass_kernel_spmd(nc, in_maps, core_ids=list(range(NCORES)),
                               trace=_trace)

    out = np.zeros((B, NUM_HEADS, D), dtype=np.float32)
    for c in range(NCORES):
        oc = res.results[c]["out"]  # [SLOTS, P, 32]  (out^T per slot)
        for s in range(SLOTS):
            b = int(assign[c, s])
            out[b] = oc[s].transpose(1, 0)
    out = out.reshape(B, NUM_HEADS * D)
    if _trace:
        return out, res
    return out


# revision 9
# speedup vs baseline: 2.0376x; 1.0250x over previous
"""Trainium2 Bass kernel: paged int8-KV-cache GQA decode attention, 8-core SPMD.

Contract: kernel(**inputs) takes the FULL unsharded numpy inputs (as produced by
the reference setup_inputs) and returns the FULL [32, 4096] float32 output.

Strategy (pure data parallel over sequences, per the sharding hint):
  - 32 decode sequences are sorted by context length and dealt across the
    8 cores (one per length-octile slot), so every core owns 4 sequences and
    runs an identical, statically-shaped program.
  - Host staging is permutation/layout only: the int8-valued int32 KV cache is
    gathered per block_tables into per-core packed **int8** buffers (K
    transposed to [kvh, d, tokens], V natural [kvh, tokens, d]).  The new
    decode token is quantized and scattered exactly as the reference's
    store_kvcache does, before the gather.
  - On device, SWDGE DMAs cast int8 -> bf16 inline during the HBM->SBUF load
    (4x less HBM traffic than the int32 container); k_scale * softmax_scale
    and v_scale are folded in as per-token vectors after the QK matmul / after
    exp respectively.
  - Per (slot, group of 4 kv heads):
      scores [128t, 4kvh, n, 4h] = per-tile matmuls(lhsT=K^T tile, rhs=q^T)
      s1 = scores * ksb  (DVE; ksb = k_scale*SCALE, zeroed beyond ctx)
      e  = exp(s1) in bf16 (ACT), one op per 4-kvh group
      ev = e * v_scale_vec (DVE; v_scale zeroed beyond ctx)
      Z  = matmul(lhsT=e, rhs=ones) per kvh -> fold to [1, 32]; the pad
           region contributes exp(0)=1 per token, corrected by subtracting a
           host-computed pad count (no mask multiply needed)
      PV = matmul(lhsT=V tile [128t,128d], rhs=ev [128t,4]) accumulated in
           PSUM as out^T [128d, 4h]; at slot end out = pv * (1/Z) with 1/Z
           broadcast across partitions via a rank-1 matmul.
  Softmax skips max-subtraction (scores are O(20) at most; fp32 exp is safe).
"""

import os
import sys
import math
from contextlib import ExitStack

import numpy as np

sys.path.insert(0, "/opt/trn_rl_repo")

import ml_dtypes  # noqa: E402

import concourse.bass as bass  # noqa: E402
import concourse.mybir as mybir  # noqa: E402
import concourse.tile as tile  # noqa: E402
from concourse import bacc  # noqa: E402
from concourse.bass_utils import run_bass_kernel_spmd  # noqa: E402

BF16 = ml_dtypes.bfloat16

B = 32
NUM_HEADS = 32
KVH = 8
D = 128
REP = NUM_HEADS // KVH  # 4
BLOCK_SIZE = 256
T = 4096
P = 128
SCALE = 1.0 / float(np.sqrt(D))
NCORES = 8
SLOTS = 4


# ---------------------------------------------------------------------------
# host-side planning + packing
# ---------------------------------------------------------------------------

def _plan(context_lens):
    """Assign sequences to (core, slot); slot tile counts = octile maxima."""
    order = np.argsort(-context_lens, kind="stable")  # descending
    ns = []
    assign = np.zeros((NCORES, SLOTS), dtype=np.int64)
    for s in range(SLOTS):
        octile = order[8 * s: 8 * s + 8]
        ns.append(int(math.ceil(int(context_lens[octile[0]]) / P)))
        # alternate direction per slot to roughly balance true work
        ranks = octile if s % 2 == 0 else octile[::-1]
        for c in range(NCORES):
            assign[c, s] = ranks[c]
    return assign, ns


def _quantize(x):
    absmax = np.abs(x).max(axis=-1)
    scale = np.where(absmax > 0.0, absmax / 127.0, 1.0).astype(np.float32)
    xq = np.clip(np.round(x / scale[..., None]), -127.0, 127.0).astype(np.int32)
    return xq, scale


def _pack_inputs(inputs, assign, ns):
    q = inputs["q"].reshape(B, NUM_HEADS, D).astype(np.float32)
    k = inputs["k"].reshape(B, KVH, D).astype(np.float32)
    v = inputs["v"].reshape(B, KVH, D).astype(np.float32)
    kc = np.ascontiguousarray(inputs["k_cache_q"].reshape(-1, KVH, D))
    vc = np.ascontiguousarray(inputs["v_cache_q"].reshape(-1, KVH, D))
    ks = np.ascontiguousarray(inputs["k_scale"].reshape(-1, KVH)).astype(np.float32)
    vs = np.ascontiguousarray(inputs["v_scale"].reshape(-1, KVH)).astype(np.float32)
    bt = inputs["block_tables"]
    ctx = inputs["context_lens"]
    sm = inputs["slot_mapping"]

    # store_kvcache_int8: quantize the new token and scatter into the cache
    kq, ksn = _quantize(k)
    vq, vsn = _quantize(v)
    kc = kc.copy(); vc = vc.copy(); ks = ks.copy(); vs = vs.copy()
    kc[sm] = kq; vc[sm] = vq; ks[sm] = ksn; vs[sm] = vsn

    NTT = sum(ns)           # token tiles per core
    NT = NTT * P            # tokens per core
    offs = np.concatenate([[0], np.cumsum(ns)])

    in_maps = []
    for c in range(NCORES):
        # K and V are packed slot-major so each (slot, 4-kvh-group) load is
        # ONE contiguous DRAM block whose element order matches the SBUF tile
        # free-dim order => 128 big descriptors per DMA (one per partition).
        # K block: [d, j2, tokens]; V block: [p, j2, tile, d].
        kt_c = np.zeros((1, KVH * D * NT), dtype=np.int8)
        vp_c = np.zeros((1, KVH * P * NTT * D), dtype=np.int8)
        # scales per slot, kvh-major then tile: [P, kvh, tiles]
        ksb_c = np.zeros((P, KVH * NTT), dtype=np.float32)
        vsb_c = np.zeros((P, KVH * NTT), dtype=BF16)
        qt_c = np.zeros((P, SLOTS * 32), dtype=np.float32)
        padc_c = np.zeros((1, SLOTS), dtype=np.float32)
        for s in range(SLOTS):
            b = int(assign[c, s])
            n = ns[s]
            nt = n * P
            o = int(offs[s])
            padc_c[0, s] = float(nt - int(ctx[b]))
            flat = (bt[b][:, None] * BLOCK_SIZE
                    + np.arange(BLOCK_SIZE, dtype=np.int64)[None, :]).reshape(-1)[:nt]
            kg = kc[flat]                      # [nt, KVH, D] int32 (int8 vals)
            vg = vc[flat]
            kjdt = kg.transpose(1, 2, 0)       # [KVH, D, nt]
            vpjid = vg.reshape(n, P, KVH, D).transpose(1, 2, 0, 3)  # [P,KVH,n,D]
            for g in range(KVH // 4):
                ko = (8 * o * D * P + g * 4 * D * nt)
                kt_c[0, ko: ko + 4 * D * nt] = (
                    kjdt[4 * g: 4 * g + 4].transpose(1, 0, 2).reshape(-1))
                vo = (8 * o * P * D + g * 4 * P * n * D)
                vp_c[0, vo: vo + 4 * P * n * D] = (
                    vpjid[:, 4 * g: 4 * g + 4].reshape(-1))
            valid = (np.arange(nt) < int(ctx[b]))
            ksg = (ks[flat] * SCALE) * valid[:, None]        # [nt, KVH]
            vsg = vs[flat] * valid[:, None]
            # [nt, KVH] -> [P, KVH * n]  (kvh-major; head-broadcast on device)
            def sprd(a, dt):
                return a.reshape(n, P, KVH).transpose(1, 2, 0).reshape(
                    P, KVH * n).astype(dt)
            ksb_c[:, o * KVH: (o + n) * KVH] = sprd(ksg, np.float32)
            vsb_c[:, o * KVH: (o + n) * KVH] = sprd(vsg, BF16)
            qt_c[:, s * 32: (s + 1) * 32] = q[b].transpose(1, 0)  # [D, 32]
        sel = np.tile(np.eye(4, dtype=np.float32), (32, 1))       # [128, 4]
        in_maps.append(dict(kt=kt_c, vp=vp_c, ksb=ksb_c, vsb=vsb_c,
                            qt=qt_c, sel=sel, padc=padc_c))
    return in_maps


# ---------------------------------------------------------------------------
# device program
# ---------------------------------------------------------------------------

def _build_program(ns):
    NTT = sum(ns)
    NT = NTT * P
    offs = [0]
    for n in ns:
        offs.append(offs[-1] + n)
    f32 = mybir.dt.float32
    bf16 = mybir.dt.bfloat16
    i8 = mybir.dt.int8
    EXP = mybir.ActivationFunctionType.Exp

    nc = bacc.Bacc("TRN2", target_bir_lowering=False, debug=False,
                   num_devices=NCORES)

    kt_d = nc.dram_tensor("kt", [1, KVH * D * NT], i8, kind="ExternalInput").ap()
    vp_d = nc.dram_tensor("vp", [1, KVH * P * NTT * D], i8,
                          kind="ExternalInput").ap()
    ksb_d = nc.dram_tensor("ksb", [P, KVH * NTT], f32, kind="ExternalInput").ap()
    vsb_d = nc.dram_tensor("vsb", [P, KVH * NTT], bf16, kind="ExternalInput").ap()
    qt_d = nc.dram_tensor("qt", [P, SLOTS * 32], f32, kind="ExternalInput").ap()
    sel_d = nc.dram_tensor("sel", [P, 4], f32, kind="ExternalInput").ap()
    padc_d = nc.dram_tensor("padc", [1, SLOTS], f32, kind="ExternalInput").ap()
    out_d = nc.dram_tensor("out", [SLOTS, P, 32], f32,
                           kind="ExternalOutput").ap()

    with tile.TileContext(nc) as tc, ExitStack() as ctx:
        const = ctx.enter_context(tc.tile_pool(name="const", bufs=1))
        kt_pool = ctx.enter_context(tc.tile_pool(name="ktp", bufs=2))
        v_pool = ctx.enter_context(tc.tile_pool(name="vpp", bufs=2))
        sc_pool = ctx.enter_context(tc.tile_pool(name="scp", bufs=2))
        work = ctx.enter_context(tc.tile_pool(name="wrk", bufs=3))
        tail = ctx.enter_context(tc.tile_pool(name="tl", bufs=2))
        ps_qk = ctx.enter_context(tc.tile_pool(name="psqk", bufs=2, space="PSUM"))
        ps_pt = ctx.enter_context(tc.tile_pool(name="pspt", bufs=2, space="PSUM"))
        ps_z = ctx.enter_context(tc.tile_pool(name="psz", bufs=1, space="PSUM"))
        ps_pv = ctx.enter_context(tc.tile_pool(name="pspv", bufs=2, space="PSUM"))
        ps_zb = ctx.enter_context(tc.tile_pool(name="pszb", bufs=1, space="PSUM"))

        qt_f = const.tile([P, SLOTS * 32], f32)
        nc.sync.dma_start(qt_f, qt_d)
        qt = const.tile([P, SLOTS * 32], bf16)
        nc.vector.tensor_copy(qt, qt_f)
        sel = const.tile([P, 4], f32)
        nc.sync.dma_start(sel, sel_d)
        padc = const.tile([1, SLOTS], f32)
        nc.sync.dma_start(padc, padc_d)
        ones = const.tile([P, 1], bf16)
        nc.vector.memset(ones, 1.0)
        ones1 = const.tile([1, P], f32)
        nc.vector.memset(ones1, 1.0)

        for s in range(SLOTS):
            n = ns[s]
            o = offs[s]
            ksb_s = sc_pool.tile([P, KVH, n, 1], f32, tag="ksb")
            nc.sync.dma_start(ksb_s, ksb_d[:, o * KVH: (o + n) * KVH])
            vsb_s = sc_pool.tile([P, KVH, n, 1], bf16, tag="vsb")
            nc.sync.dma_start(vsb_s, vsb_d[:, o * KVH: (o + n) * KVH])

            pv = ps_pv.tile([P, 32], f32, tag="pv")
            pt = ps_pt.tile([P, KVH], f32, tag="pt")
            z_all = ps_z.tile([1, 32], f32, tag="z")

            for jq in range(KVH // 4):
                ktc = kt_pool.tile([P, 4, n, P], bf16, tag="kt")
                vtc = v_pool.tile([P, 4, n, D], bf16, tag="vt")
                ko = 8 * o * D * P + jq * 4 * D * n * P
                nc.gpsimd.dma_start(
                    ktc,
                    kt_d[0:1, ko: ko + 4 * D * n * P].rearrange(
                        "o (d r) -> (o d) r", d=P))
                vo = 8 * o * P * D + jq * 4 * P * n * D
                nc.gpsimd.dma_start(
                    vtc,
                    vp_d[0:1, vo: vo + 4 * P * n * D].rearrange(
                        "o (p r) -> (o p) r", p=P))

                # QK for the 4 kv heads of this group
                qk = ps_qk.tile([P, 4, n, 4], f32, tag="qk")
                for j2 in range(4):
                    qcol = s * 32 + 4 * (4 * jq + j2)
                    for i in range(n):
                        nc.tensor.matmul(
                            qk[:, j2, i, :],
                            lhsT=ktc[:, j2, i, :],
                            rhs=qt[:, qcol: qcol + 4],
                            start=True, stop=True, skip_group_check=True)

                # softmax pieces, batched over the 4-kvh group
                s1 = work.tile([P, 4, n, 4], f32, tag="s1")
                nc.vector.tensor_mul(
                    s1, qk,
                    ksb_s[:, 4 * jq: 4 * jq + 4].to_broadcast([P, 4, n, 4]))
                e = work.tile([P, 4, n, 4], bf16, tag="e")
                nc.scalar.activation(e, s1, EXP)
                ev = work.tile([P, 4, n, 4], bf16, tag="ev")
                nc.vector.tensor_mul(
                    ev, e,
                    vsb_s[:, 4 * jq: 4 * jq + 4].to_broadcast([P, 4, n, 4]))

                for j2 in range(4):
                    j = 4 * jq + j2
                    # Z partials: per-(tile, head) column sums of e
                    nc.tensor.matmul(
                        pt[0: n * 4, j: j + 1],
                        lhsT=e[:, j2], rhs=ones,
                        start=True, stop=True, skip_group_check=True)
                    # PV accumulate over token tiles: out^T [128d, 4h]
                    cc = 4 * j
                    for i in range(n):
                        nc.tensor.matmul(
                            pv[:, cc: cc + 4],
                            lhsT=vtc[:, j2, i, :],
                            rhs=ev[:, j2, i, :],
                            start=(i == 0), stop=(i == n - 1),
                            skip_group_check=True)

            # fold Z partials -> [1, 32], correct for exp(0)=1 pad terms
            pts = tail.tile([P, KVH], f32, tag="pts")
            nc.vector.tensor_copy(pts[0: n * 4, :], pt[0: n * 4, :])
            for j in range(KVH):
                nc.tensor.matmul(
                    z_all[0:1, 4 * j: 4 * j + 4],
                    lhsT=pts[0: n * 4, j: j + 1],
                    rhs=sel[0: n * 4, :],
                    start=True, stop=True, skip_group_check=True)
            zs = tail.tile([1, 32], f32, tag="zs")
            nc.vector.tensor_scalar_sub(zs, z_all, padc[0:1, s: s + 1])
            rz = tail.tile([1, 32], f32, tag="rz")
            nc.vector.reciprocal(rz, zs)
            # broadcast 1/Z across partitions with a rank-1 matmul
            zb = ps_zb.tile([P, 32], f32, tag="zb")
            nc.tensor.matmul(zb, lhsT=ones1, rhs=rz, start=True, stop=True)
            zbs = tail.tile([P, 32], f32, tag="zbs")
            nc.vector.tensor_copy(zbs, zb)
            o_sb = tail.tile([P, 32], f32, tag="o")
            nc.vector.tensor_mul(o_sb, pv, zbs)
            nc.sync.dma_start(out_d[s], o_sb)

    nc.compile()
    return nc


_PROGRAM_CACHE = {}


def _get_program(ns):
    key = tuple(ns)
    if key not in _PROGRAM_CACHE:
        _PROGRAM_CACHE[key] = _build_program(ns)
    return _PROGRAM_CACHE[key]


# ---------------------------------------------------------------------------
# entry point
# ---------------------------------------------------------------------------

def kernel(q, k, v, k_cache_q, v_cache_q, k_scale, v_scale,
           block_tables, context_lens, slot_mapping, _trace=False):
    inputs = dict(q=np.asarray(q), k=np.asarray(k), v=np.asarray(v),
                  k_cache_q=np.asarray(k_cache_q),
                  v_cache_q=np.asarray(v_cache_q),
                  k_scale=np.asarray(k_scale), v_scale=np.asarray(v_scale),
                  block_tables=np.asarray(block_tables),
                  context_lens=np.asarray(context_lens),
                  slot_mapping=np.asarray(slot_mapping))
    assign, ns = _plan(inputs["context_lens"])
    in_maps = _pack_inputs(inputs, assign, ns)
    nc = _get_program(ns)
    res = run_bass_kernel_spmd(nc, in_maps, core_ids=list(range(NCORES)),
                               trace=_trace)

    out = np.zeros((B, NUM_HEADS, D), dtype=np.float32)
    for c in range(NCORES):
        oc = res.results[c]["out"]  # [SLOTS, P, 32]  (out^T per slot)
        for s in range(SLOTS):
            b = int(assign[c, s])
            out[b] = oc[s].transpose(1, 0)
    out = out.reshape(B, NUM_HEADS * D)
    if _trace:
        return out, res
    return out


# revision 11
# speedup vs baseline: 2.0511x; 1.0066x over previous
"""Trainium2 Bass kernel: paged int8-KV-cache GQA decode attention, 8-core SPMD.

Contract: kernel(**inputs) takes the FULL unsharded numpy inputs (as produced by
the reference setup_inputs) and returns the FULL [32, 4096] float32 output.

Strategy (data parallel over sequence-chunks, flash-decoding style):
  - The 32 sequences' token tiles (ceil(ctx/128) each) are carved into
    8 cores x SLOTS contiguous chunks; slot s has a fixed tile count L[s]
    shared by all cores (SPMD), chosen by a small search to minimize padding
    (sum(L) ~ 6% over the ideal total/8).  A long sequence may span chunks on
    several cores; every chunk computes unnormalized partials (PV^T, Z) and
    the host combines: out = sum(PV) / sum(Z) (softmax without max-shift is
    linear in the partials).
  - Host staging is permutation/layout only: the int8-valued int32 KV cache is
    gathered per block_tables into per-core packed int8 buffers, slot-major so
    each (slot, 4-kvh-group) load is one contiguous DRAM block (128 big DMA
    descriptors).  The new decode token is quantized and scattered exactly as
    the reference's store_kvcache does, before the gather.
  - On device, SWDGE DMAs cast int8 -> bf16 inline during the HBM->SBUF load
    (the DMA engines charge max(src,dst) bytes, so this hits the bf16-landing
    floor of ~45 MB/core); k_scale * softmax_scale and v_scale are folded in
    as per-token vectors after the QK matmul / after exp respectively.
  - Per (slot, group of 4 kv heads):
      scores [128t, 4kvh, n, 4h] = per-tile matmuls(lhsT=K^T tile, rhs=q^T)
      s1 = scores * ksb  (DVE; ksb = k_scale*SCALE, zeroed beyond ctx)
      e  = exp(s1) in bf16 (ACT), one op per 4-kvh group
      ev = e * v_scale_vec (DVE; v_scale zeroed beyond ctx)
      Z  = matmul(lhsT=e, rhs=ones) per kvh -> fold to [1, 32]; pad tokens
           contribute exp(0)=1 each, corrected host-side via the known count
      PV = matmul(lhsT=V tile [128t,128d], rhs=ev [128t,4]) accumulated in
           PSUM as out^T [128d, 4h].
  Softmax skips max-subtraction (scores are O(20) at most; fp32 exp is safe).
"""

import os
import sys
import math
from contextlib import ExitStack

import numpy as np

sys.path.insert(0, "/opt/trn_rl_repo")

import ml_dtypes  # noqa: E402

import concourse.bass as bass  # noqa: E402
import concourse.mybir as mybir  # noqa: E402
import concourse.tile as tile  # noqa: E402
from concourse import bacc  # noqa: E402
from concourse.bass_utils import run_bass_kernel_spmd  # noqa: E402

BF16 = ml_dtypes.bfloat16

B = 32
NUM_HEADS = 32
KVH = 8
D = 128
REP = NUM_HEADS // KVH  # 4
BLOCK_SIZE = 256
T = 4096
P = 128
SCALE = 1.0 / float(np.sqrt(D))
NCORES = 8


# ---------------------------------------------------------------------------
# host-side planning + packing
# ---------------------------------------------------------------------------

def _greedy_chunks(tiles, L):
    """Slot-by-slot, give the 8 largest remaining sequences a chunk of up to
    L[s] tiles.  Returns per-slot lists of (seq, start_tile, len) or None if
    some sequence is left uncovered."""
    rem = [int(t) for t in tiles]
    start = [0] * len(tiles)
    chunks = []
    for Ls in L:
        order = sorted(range(len(tiles)), key=lambda b: -rem[b])
        sc = []
        for c in range(NCORES):
            b = order[c]
            ln = min(rem[b], Ls)
            sc.append((b, start[b], ln))
            rem[b] -= ln
            start[b] += ln
        chunks.append(sc)
    if any(r > 0 for r in rem):
        return None
    return chunks


_PLAN_CACHE = {}


def _plan(context_lens):
    """Choose slot lengths L and the (core, slot) -> sequence-chunk map."""
    tiles = tuple(int(math.ceil(int(c) / P)) for c in context_lens)
    if tiles in _PLAN_CACHE:
        return _PLAN_CACHE[tiles]
    ts = sorted(tiles, reverse=True)
    # octile fallback (always feasible): whole sequences, 4 slots
    best = (ts[0] + ts[8] + ts[16] + ts[24], (ts[0], ts[8], ts[16], ts[24]))
    for L0 in range(max(4, ts[0] - 8), ts[0] + 1):
        for L1 in range(max(4, ts[8] - 8), min(L0, ts[8] + 4) + 1):
            for L2 in range(max(4, ts[16] - 6), min(L1, ts[16] + 4) + 1):
                for L3 in range(max(4, ts[24] - 4), min(L2, ts[24] + 4) + 1):
                    for L4 in range(4, min(L3, 10) + 1):
                        for L5 in (0, *range(4, L4 + 1)):
                            L = (L0, L1, L2, L3, L4) if L5 == 0 else (
                                L0, L1, L2, L3, L4, L5)
                            N = sum(L)
                            if N >= best[0]:
                                continue
                            if _greedy_chunks(tiles, L) is not None:
                                best = (N, L)
    L = list(best[1])
    chunks = _greedy_chunks(tiles, L)
    _PLAN_CACHE[tiles] = (L, chunks)
    return L, chunks


def _quantize(x):
    absmax = np.abs(x).max(axis=-1)
    scale = np.where(absmax > 0.0, absmax / 127.0, 1.0).astype(np.float32)
    xq = np.clip(np.round(x / scale[..., None]), -127.0, 127.0).astype(np.int32)
    return xq, scale


def _pack_inputs(inputs, L, chunks):
    q = inputs["q"].reshape(B, NUM_HEADS, D).astype(np.float32)
    k = inputs["k"].reshape(B, KVH, D).astype(np.float32)
    v = inputs["v"].reshape(B, KVH, D).astype(np.float32)
    kc = np.ascontiguousarray(inputs["k_cache_q"].reshape(-1, KVH, D))
    vc = np.ascontiguousarray(inputs["v_cache_q"].reshape(-1, KVH, D))
    ks = np.ascontiguousarray(inputs["k_scale"].reshape(-1, KVH)).astype(np.float32)
    vs = np.ascontiguousarray(inputs["v_scale"].reshape(-1, KVH)).astype(np.float32)
    bt = inputs["block_tables"]
    ctx = inputs["context_lens"]
    sm = inputs["slot_mapping"]

    # store_kvcache_int8: quantize the new token and scatter into the cache
    kq, ksn = _quantize(k)
    vq, vsn = _quantize(v)
    kc = kc.copy(); vc = vc.copy(); ks = ks.copy(); vs = vs.copy()
    kc[sm] = kq; vc[sm] = vq; ks[sm] = ksn; vs[sm] = vsn

    SLOTS = len(L)
    NTT = sum(L)
    NT = NTT * P
    offs = np.concatenate([[0], np.cumsum(L)])

    in_maps = []
    padcnt = np.zeros((NCORES, SLOTS), dtype=np.float64)
    for c in range(NCORES):
        # K block per (slot, 4-kvh group): [d, j2, tokens]; V: [p, j2, tile, d]
        kt_c = np.zeros((1, KVH * D * NT), dtype=np.int8)
        vp_c = np.zeros((1, KVH * P * NTT * D), dtype=np.int8)
        ksb_c = np.zeros((P, KVH * NTT), dtype=np.float32)
        vsb_c = np.zeros((P, KVH * NTT), dtype=BF16)
        qt_c = np.zeros((P, SLOTS * 32), dtype=np.float32)
        for s in range(SLOTS):
            b, t0, ln = chunks[s][c]
            n = L[s]
            nt = n * P
            o = int(offs[s])
            nvalid = max(0, min(int(ctx[b]) - t0 * P, ln * P))
            padcnt[c, s] = nt - nvalid
            if ln > 0:
                flat = (bt[b][:, None] * BLOCK_SIZE
                        + np.arange(BLOCK_SIZE, dtype=np.int64)[None, :]
                        ).reshape(-1)[t0 * P: t0 * P + ln * P]
                kg = np.zeros((nt, KVH, D), dtype=np.int8)
                vg = np.zeros((nt, KVH, D), dtype=np.int8)
                kg[: ln * P] = kc[flat]
                vg[: ln * P] = vc[flat]
                scg = np.zeros((nt, KVH), dtype=np.float32)
                svg = np.zeros((nt, KVH), dtype=np.float32)
                valid = (np.arange(nt) < nvalid)
                scg[: ln * P] = ks[flat] * SCALE
                svg[: ln * P] = vs[flat]
                scg *= valid[:, None]
                svg *= valid[:, None]
                kjdt = kg.transpose(1, 2, 0)                      # [KVH, D, nt]
                vpjid = vg.reshape(n, P, KVH, D).transpose(1, 2, 0, 3)
                for g in range(KVH // 4):
                    ko = 8 * o * D * P + g * 4 * D * nt
                    kt_c[0, ko: ko + 4 * D * nt] = (
                        kjdt[4 * g: 4 * g + 4].transpose(1, 0, 2).reshape(-1))
                    vo = 8 * o * P * D + g * 4 * P * n * D
                    vp_c[0, vo: vo + 4 * P * n * D] = (
                        vpjid[:, 4 * g: 4 * g + 4].reshape(-1))

                def sprd(a, dt):
                    return a.reshape(n, P, KVH).transpose(1, 2, 0).reshape(
                        P, KVH * n).astype(dt)
                ksb_c[:, o * KVH: (o + n) * KVH] = sprd(scg, np.float32)
                vsb_c[:, o * KVH: (o + n) * KVH] = sprd(svg, BF16)
            qt_c[:, s * 32: (s + 1) * 32] = q[b].transpose(1, 0)  # [D, 32]
        sel = np.tile(np.eye(4, dtype=np.float32), (32, 1))       # [128, 4]
        in_maps.append(dict(kt=kt_c, vp=vp_c, ksb=ksb_c, vsb=vsb_c,
                            qt=qt_c, sel=sel))
    return in_maps, padcnt


# ---------------------------------------------------------------------------
# device program
# ---------------------------------------------------------------------------

def _build_program(L):
    SLOTS = len(L)
    NTT = sum(L)
    NT = NTT * P
    offs = [0]
    for n in L:
        offs.append(offs[-1] + n)
    f32 = mybir.dt.float32
    bf16 = mybir.dt.bfloat16
    i8 = mybir.dt.int8
    EXP = mybir.ActivationFunctionType.Exp

    nc = bacc.Bacc("TRN2", target_bir_lowering=False, debug=False,
                   num_devices=NCORES)

    kt_d = nc.dram_tensor("kt", [1, KVH * D * NT], i8, kind="ExternalInput").ap()
    vp_d = nc.dram_tensor("vp", [1, KVH * P * NTT * D], i8,
                          kind="ExternalInput").ap()
    ksb_d = nc.dram_tensor("ksb", [P, KVH * NTT], f32, kind="ExternalInput").ap()
    vsb_d = nc.dram_tensor("vsb", [P, KVH * NTT], bf16, kind="ExternalInput").ap()
    qt_d = nc.dram_tensor("qt", [P, SLOTS * 32], f32, kind="ExternalInput").ap()
    sel_d = nc.dram_tensor("sel", [P, 4], f32, kind="ExternalInput").ap()
    pv_d = nc.dram_tensor("pv", [SLOTS, P, 32], f32, kind="ExternalOutput").ap()
    z_d = nc.dram_tensor("z", [SLOTS, 1, 32], f32, kind="ExternalOutput").ap()

    with tile.TileContext(nc) as tc, ExitStack() as ctx:
        const = ctx.enter_context(tc.tile_pool(name="const", bufs=1))
        kt_pool = ctx.enter_context(tc.tile_pool(name="ktp", bufs=3))
        v_pool = ctx.enter_context(tc.tile_pool(name="vpp", bufs=3))
        sc_pool = ctx.enter_context(tc.tile_pool(name="scp", bufs=2))
        work = ctx.enter_context(tc.tile_pool(name="wrk", bufs=3))
        tail = ctx.enter_context(tc.tile_pool(name="tl", bufs=2))
        ps_qk = ctx.enter_context(tc.tile_pool(name="psqk", bufs=3, space="PSUM"))
        ps_pt = ctx.enter_context(tc.tile_pool(name="pspt", bufs=2, space="PSUM"))
        ps_z = ctx.enter_context(tc.tile_pool(name="psz", bufs=1, space="PSUM"))
        ps_pv = ctx.enter_context(tc.tile_pool(name="pspv", bufs=2, space="PSUM"))

        qt_f = const.tile([P, SLOTS * 32], f32)
        nc.sync.dma_start(qt_f, qt_d)
        qt = const.tile([P, SLOTS * 32], bf16)
        nc.vector.tensor_copy(qt, qt_f)
        sel = const.tile([P, 4], f32)
        nc.sync.dma_start(sel, sel_d)
        ones = const.tile([P, 1], bf16)
        nc.vector.memset(ones, 1.0)

        for s in range(SLOTS):
            n = L[s]
            o = offs[s]
            ksb_s = sc_pool.tile([P, KVH, n, 1], f32, tag="ksb")
            nc.sync.dma_start(ksb_s, ksb_d[:, o * KVH: (o + n) * KVH])
            vsb_s = sc_pool.tile([P, KVH, n, 1], bf16, tag="vsb")
            nc.sync.dma_start(vsb_s, vsb_d[:, o * KVH: (o + n) * KVH])

            pv = ps_pv.tile([P, 32], f32, tag="pv")
            pt = ps_pt.tile([P, KVH], f32, tag="pt")
            z_all = ps_z.tile([1, 32], f32, tag="z")

            for jq in range(KVH // 4):
                ktc = kt_pool.tile([P, 4, n, P], bf16, tag="kt")
                vtc = v_pool.tile([P, 4, n, D], bf16, tag="vt")
                ko = 8 * o * D * P + jq * 4 * D * n * P
                nc.gpsimd.dma_start(
                    ktc,
                    kt_d[0:1, ko: ko + 4 * D * n * P].rearrange(
                        "o (d r) -> (o d) r", d=P))
                vo = 8 * o * P * D + jq * 4 * P * n * D
                nc.gpsimd.dma_start(
                    vtc,
                    vp_d[0:1, vo: vo + 4 * P * n * D].rearrange(
                        "o (p r) -> (o p) r", p=P))

                # QK for the 4 kv heads of this group
                qk = ps_qk.tile([P, 4, n, 4], f32, tag="qk")
                for j2 in range(4):
                    qcol = s * 32 + 4 * (4 * jq + j2)
                    for i in range(n):
                        nc.tensor.matmul(
                            qk[:, j2, i, :],
                            lhsT=ktc[:, j2, i, :],
                            rhs=qt[:, qcol: qcol + 4],
                            start=True, stop=True, skip_group_check=True)

                # softmax pieces, batched over the 4-kvh group
                s1 = work.tile([P, 4, n, 4], f32, tag="s1")
                nc.vector.tensor_mul(
                    s1, qk,
                    ksb_s[:, 4 * jq: 4 * jq + 4].to_broadcast([P, 4, n, 4]))
                e = work.tile([P, 4, n, 4], bf16, tag="e")
                nc.scalar.activation(e, s1, EXP)
                ev = work.tile([P, 4, n, 4], bf16, tag="ev")
                nc.vector.tensor_mul(
                    ev, e,
                    vsb_s[:, 4 * jq: 4 * jq + 4].to_broadcast([P, 4, n, 4]))

                for j2 in range(4):
                    j = 4 * jq + j2
                    # Z partials: per-(tile, head) column sums of e
                    nc.tensor.matmul(
                        pt[0: n * 4, j: j + 1],
                        lhsT=e[:, j2], rhs=ones,
                        start=True, stop=True, skip_group_check=True)
                    # PV accumulate over token tiles: out^T [128d, 4h]
                    cc = 4 * j
                    for i in range(n):
                        nc.tensor.matmul(
                            pv[:, cc: cc + 4],
                            lhsT=vtc[:, j2, i, :],
                            rhs=ev[:, j2, i, :],
                            start=(i == 0), stop=(i == n - 1),
                            skip_group_check=True)

            # fold Z partials -> [1, 32]; ship unnormalized partials to host
            pts = tail.tile([P, KVH], f32, tag="pts")
            nc.vector.tensor_copy(pts[0: n * 4, :], pt[0: n * 4, :])
            for j in range(KVH):
                nc.tensor.matmul(
                    z_all[0:1, 4 * j: 4 * j + 4],
                    lhsT=pts[0: n * 4, j: j + 1],
                    rhs=sel[0: n * 4, :],
                    start=True, stop=True, skip_group_check=True)
            zs = tail.tile([1, 32], f32, tag="zs")
            nc.vector.tensor_copy(zs, z_all)
            nc.sync.dma_start(z_d[s], zs)
            pvs = tail.tile([P, 32], f32, tag="pvs")
            nc.vector.tensor_copy(pvs, pv)
            nc.sync.dma_start(pv_d[s], pvs)

    nc.compile()
    return nc


_PROGRAM_CACHE = {}


def _get_program(L):
    key = tuple(L)
    if key not in _PROGRAM_CACHE:
        _PROGRAM_CACHE[key] = _build_program(L)
    return _PROGRAM_CACHE[key]


# ---------------------------------------------------------------------------
# entry point
# ---------------------------------------------------------------------------

def kernel(q, k, v, k_cache_q, v_cache_q, k_scale, v_scale,
           block_tables, context_lens, slot_mapping, _trace=False):
    inputs = dict(q=np.asarray(q), k=np.asarray(k), v=np.asarray(v),
                  k_cache_q=np.asarray(k_cache_q),
                  v_cache_q=np.asarray(v_cache_q),
                  k_scale=np.asarray(k_scale), v_scale=np.asarray(v_scale),
                  block_tables=np.asarray(block_tables),
                  context_lens=np.asarray(context_lens),
                  slot_mapping=np.asarray(slot_mapping))
    L, chunks = _plan(inputs["context_lens"])
    in_maps, padcnt = _pack_inputs(inputs, L, chunks)
    nc = _get_program(L)
    res = run_bass_kernel_spmd(nc, in_maps, core_ids=list(range(NCORES)),
                               trace=_trace)

    # combine unnormalized partials across chunks (flash-decoding merge)
    accp = np.zeros((B, P, 32), dtype=np.float64)
    accz = np.zeros((B, 32), dtype=np.float64)
    for c in range(NCORES):
        pvs = res.results[c]["pv"]   # [SLOTS, P, 32]
        zss = res.results[c]["z"]    # [SLOTS, 1, 32]
        for s in range(len(L)):
            b, _, _ = chunks[s][c]
            accp[b] += pvs[s]
            accz[b] += zss[s][0] - padcnt[c, s]
    out = (accp / accz[:, None, :]).transpose(0, 2, 1)  # [B, 32h, 128d]
    out = np.ascontiguousarray(out.reshape(B, NUM_HEADS * D), dtype=np.float32)
    if _trace:
        return out, res
    return out


# revision 16
# speedup vs baseline: 2.1281x; 1.0376x over previous
"""Trainium2 Bass kernel: paged int8-KV-cache GQA decode attention, 8-core SPMD.

Contract: kernel(**inputs) takes the FULL unsharded numpy inputs (as produced by
the reference setup_inputs) and returns the FULL [32, 4096] float32 output.

Strategy (data parallel over sequence-chunks, flash-decoding style):
  - The 32 sequences' token tiles (ceil(ctx/128) each) are carved into
    8 cores x SLOTS contiguous chunks; slot s has a fixed tile count L[s]
    shared by all cores (SPMD), chosen by a small search to minimize padding
    (sum(L) ~ 6% over the ideal total/8).  A long sequence may span chunks on
    several cores; every chunk computes unnormalized partials (PV^T, Z) and
    the host combines: out = sum(PV) / sum(Z) (softmax without max-shift is
    linear in the partials).
  - Host staging is permutation/layout only: the int8-valued int32 KV cache is
    gathered per block_tables into per-core packed int8 buffers, slot-major so
    each (slot, 4-kvh-group) load is one contiguous DRAM block (128 big DMA
    descriptors).  The new decode token is quantized and scattered exactly as
    the reference's store_kvcache does, before the gather.
  - On device, SWDGE DMAs cast int8 -> bf16 inline during the HBM->SBUF load
    (the DMA engines charge max(src,dst) bytes, so this hits the bf16-landing
    floor of ~45 MB/core); k_scale * softmax_scale and v_scale are folded in
    as per-token vectors after the QK matmul / after exp respectively.
  - Per (slot, group of 4 kv heads):
      scores [128t, 4kvh, n, 4h] = per-tile matmuls(lhsT=K^T tile, rhs=q^T)
      s1 = scores * ksb  (DVE; ksb = k_scale*SCALE, zeroed beyond ctx)
      e  = exp(s1) in bf16 (ACT), one op per 4-kvh group
      ev = e * v_scale_vec (DVE; v_scale zeroed beyond ctx)
      Z  = matmul(lhsT=e, rhs=ones) per kvh -> fold to [1, 32]; pad tokens
           contribute exp(0)=1 each, corrected host-side via the known count
      PV = matmul(lhsT=V tile [128t,128d], rhs=ev [128t,4]) accumulated in
           PSUM as out^T [128d, 4h].
  Softmax skips max-subtraction (scores are O(20) at most; fp32 exp is safe).
"""

import os
import sys
import math
from contextlib import ExitStack

import numpy as np

sys.path.insert(0, "/opt/trn_rl_repo")

import ml_dtypes  # noqa: E402

import concourse.bass as bass  # noqa: E402
import concourse.mybir as mybir  # noqa: E402
import concourse.tile as tile  # noqa: E402
from concourse import bacc  # noqa: E402
from concourse.bass_utils import run_bass_kernel_spmd  # noqa: E402

BF16 = ml_dtypes.bfloat16

B = 32
NUM_HEADS = 32
KVH = 8
D = 128
REP = NUM_HEADS // KVH  # 4
BLOCK_SIZE = 256
T = 4096
P = 128
SCALE = 1.0 / float(np.sqrt(D))
NCORES = 8


# ---------------------------------------------------------------------------
# host-side planning + packing
# ---------------------------------------------------------------------------

def _greedy_chunks(tiles, L):
    """Slot-by-slot, give the 8 largest remaining sequences a chunk of up to
    L[s] tiles.  Returns per-slot lists of (seq, start_tile, len) or None if
    some sequence is left uncovered."""
    rem = [int(t) for t in tiles]
    start = [0] * len(tiles)
    chunks = []
    for Ls in L:
        order = sorted(range(len(tiles)), key=lambda b: -rem[b])
        sc = []
        for c in range(NCORES):
            b = order[c]
            ln = min(rem[b], Ls)
            sc.append((b, start[b], ln))
            rem[b] -= ln
            start[b] += ln
        chunks.append(sc)
    if any(r > 0 for r in rem):
        return None
    return chunks


_PLAN_CACHE = {}


def _plan(context_lens):
    """Choose slot lengths L and the (core, slot) -> sequence-chunk map."""
    tiles = tuple(int(math.ceil(int(c) / P)) for c in context_lens)
    if tiles in _PLAN_CACHE:
        return _PLAN_CACHE[tiles]
    ts = sorted(tiles, reverse=True)
    # octile fallback (always feasible): whole sequences, 4 slots
    best = (ts[0] + ts[8] + ts[16] + ts[24], (ts[0], ts[8], ts[16], ts[24]))
    for L0 in range(max(4, ts[0] - 8), ts[0] + 1):
        for L1 in range(max(4, ts[8] - 8), min(L0, ts[8] + 4) + 1):
            for L2 in range(max(4, ts[16] - 6), min(L1, ts[16] + 4) + 1):
                for L3 in range(max(4, ts[24] - 4), min(L2, ts[24] + 4) + 1):
                    for L4 in range(4, min(L3, 10) + 1):
                        for L5 in (0, *range(4, L4 + 1)):
                            L = (L0, L1, L2, L3, L4) if L5 == 0 else (
                                L0, L1, L2, L3, L4, L5)
                            N = sum(L)
                            if N >= best[0]:
                                continue
                            if _greedy_chunks(tiles, L) is not None:
                                best = (N, L)
    L = list(best[1])
    chunks = _greedy_chunks(tiles, L)
    _PLAN_CACHE[tiles] = (L, chunks)
    return L, chunks


def _quantize(x):
    absmax = np.abs(x).max(axis=-1)
    scale = np.where(absmax > 0.0, absmax / 127.0, 1.0).astype(np.float32)
    xq = np.clip(np.round(x / scale[..., None]), -127.0, 127.0).astype(np.int32)
    return xq, scale


def _pack_inputs(inputs, L, chunks):
    q = inputs["q"].reshape(B, NUM_HEADS, D).astype(np.float32)
    k = inputs["k"].reshape(B, KVH, D).astype(np.float32)
    v = inputs["v"].reshape(B, KVH, D).astype(np.float32)
    kc = np.ascontiguousarray(inputs["k_cache_q"].reshape(-1, KVH, D))
    vc = np.ascontiguousarray(inputs["v_cache_q"].reshape(-1, KVH, D))
    ks = np.ascontiguousarray(inputs["k_scale"].reshape(-1, KVH)).astype(np.float32)
    vs = np.ascontiguousarray(inputs["v_scale"].reshape(-1, KVH)).astype(np.float32)
    bt = inputs["block_tables"]
    ctx = inputs["context_lens"]
    sm = inputs["slot_mapping"]

    # store_kvcache_int8: quantize the new token and scatter into the cache
    kq, ksn = _quantize(k)
    vq, vsn = _quantize(v)
    kc = kc.copy(); vc = vc.copy(); ks = ks.copy(); vs = vs.copy()
    kc[sm] = kq; vc[sm] = vq; ks[sm] = ksn; vs[sm] = vsn

    SLOTS = len(L)
    NTT = sum(L)
    NT = NTT * P
    offs = np.concatenate([[0], np.cumsum(L)])

    in_maps = []
    padcnt = np.zeros((NCORES, SLOTS), dtype=np.float64)
    for c in range(NCORES):
        # K block per (slot, 4-kvh group): [d, j2, tokens]; V: [p, j2, tile, d]
        kt_c = np.zeros((1, KVH * D * NT), dtype=np.int8)
        vp_c = np.zeros((1, KVH * P * NTT * D), dtype=np.int8)
        ksb_c = np.zeros((P, KVH * NTT), dtype=np.float32)
        vsb_c = np.zeros((P, KVH * NTT), dtype=BF16)
        qt_c = np.zeros((P, SLOTS * 32), dtype=np.float32)
        for s in range(SLOTS):
            b, t0, ln = chunks[s][c]
            n = L[s]
            nt = n * P
            o = int(offs[s])
            nvalid = max(0, min(int(ctx[b]) - t0 * P, ln * P))
            padcnt[c, s] = nt - nvalid
            if ln > 0:
                flat = (bt[b][:, None] * BLOCK_SIZE
                        + np.arange(BLOCK_SIZE, dtype=np.int64)[None, :]
                        ).reshape(-1)[t0 * P: t0 * P + ln * P]
                kg = np.zeros((nt, KVH, D), dtype=np.int8)
                vg = np.zeros((nt, KVH, D), dtype=np.int8)
                kg[: ln * P] = kc[flat]
                vg[: ln * P] = vc[flat]
                scg = np.zeros((nt, KVH), dtype=np.float32)
                svg = np.zeros((nt, KVH), dtype=np.float32)
                valid = (np.arange(nt) < nvalid)
                scg[: ln * P] = ks[flat] * SCALE
                svg[: ln * P] = vs[flat]
                scg *= valid[:, None]
                svg *= valid[:, None]
                kjdt = kg.transpose(1, 2, 0)                      # [KVH, D, nt]
                vpjid = vg.reshape(n, P, KVH, D).transpose(1, 2, 0, 3)
                for g in range(KVH // 2):
                    ko = 8 * o * D * P + g * 2 * D * nt
                    kt_c[0, ko: ko + 2 * D * nt] = (
                        kjdt[2 * g: 2 * g + 2].transpose(1, 0, 2).reshape(-1))
                    vo = 8 * o * P * D + g * 2 * P * n * D
                    vp_c[0, vo: vo + 2 * P * n * D] = (
                        vpjid[:, 2 * g: 2 * g + 2].reshape(-1))

                def sprd(a, dt):
                    return a.reshape(n, P, KVH).transpose(1, 2, 0).reshape(
                        P, KVH * n).astype(dt)
                ksb_c[:, o * KVH: (o + n) * KVH] = sprd(scg, np.float32)
                vsb_c[:, o * KVH: (o + n) * KVH] = sprd(svg, BF16)
            qt_c[:, s * 32: (s + 1) * 32] = q[b].transpose(1, 0)  # [D, 32]
        sel = np.tile(np.eye(4, dtype=np.float32), (32, 1))       # [128, 4]
        in_maps.append(dict(kt=kt_c, vp=vp_c, ksb=ksb_c, vsb=vsb_c,
                            qt=qt_c, sel=sel))
    return in_maps, padcnt


# ---------------------------------------------------------------------------
# device program
# ---------------------------------------------------------------------------

def _build_program(L):
    SLOTS = len(L)
    NTT = sum(L)
    NT = NTT * P
    offs = [0]
    for n in L:
        offs.append(offs[-1] + n)
    f32 = mybir.dt.float32
    bf16 = mybir.dt.bfloat16
    i8 = mybir.dt.int8
    EXP = mybir.ActivationFunctionType.Exp

    nc = bacc.Bacc("TRN2", target_bir_lowering=False, debug=False,
                   num_devices=NCORES)

    kt_d = nc.dram_tensor("kt", [1, KVH * D * NT], i8, kind="ExternalInput").ap()
    vp_d = nc.dram_tensor("vp", [1, KVH * P * NTT * D], i8,
                          kind="ExternalInput").ap()
    ksb_d = nc.dram_tensor("ksb", [P, KVH * NTT], f32, kind="ExternalInput").ap()
    vsb_d = nc.dram_tensor("vsb", [P, KVH * NTT], bf16, kind="ExternalInput").ap()
    qt_d = nc.dram_tensor("qt", [P, SLOTS * 32], f32, kind="ExternalInput").ap()
    sel_d = nc.dram_tensor("sel", [P, 4], f32, kind="ExternalInput").ap()
    pv_d = nc.dram_tensor("pv", [SLOTS, P, 32], f32, kind="ExternalOutput").ap()
    z_d = nc.dram_tensor("z", [SLOTS, 1, 32], f32, kind="ExternalOutput").ap()

    with tile.TileContext(nc) as tc, ExitStack() as ctx:
        const = ctx.enter_context(tc.tile_pool(name="const", bufs=1))
        kt_pool = ctx.enter_context(tc.tile_pool(name="ktp", bufs=5))
        v_pool = ctx.enter_context(tc.tile_pool(name="vpp", bufs=5))
        sc_pool = ctx.enter_context(tc.tile_pool(name="scp", bufs=3))
        work = ctx.enter_context(tc.tile_pool(name="wrk", bufs=3))
        tail = ctx.enter_context(tc.tile_pool(name="tl", bufs=2))
        ps_qk = ctx.enter_context(tc.tile_pool(name="psqk", bufs=3, space="PSUM"))
        ps_pt = ctx.enter_context(tc.tile_pool(name="pspt", bufs=2, space="PSUM"))
        ps_z = ctx.enter_context(tc.tile_pool(name="psz", bufs=1, space="PSUM"))
        ps_pv = ctx.enter_context(tc.tile_pool(name="pspv", bufs=2, space="PSUM"))

        qt_f = const.tile([P, SLOTS * 32], f32)
        nc.sync.dma_start(qt_f, qt_d)
        qt = const.tile([P, SLOTS * 32], bf16)
        nc.vector.tensor_copy(qt, qt_f)
        sel = const.tile([P, 4], f32)
        nc.sync.dma_start(sel, sel_d)
        ones = const.tile([P, 1], bf16)
        nc.vector.memset(ones, 1.0)

        for s in range(SLOTS):
            n = L[s]
            o = offs[s]
            ksb_s = sc_pool.tile([P, KVH, n, 1], f32, tag="ksb")
            nc.sync.dma_start(ksb_s, ksb_d[:, o * KVH: (o + n) * KVH])
            vsb_s = sc_pool.tile([P, KVH, n, 1], bf16, tag="vsb")
            nc.sync.dma_start(vsb_s, vsb_d[:, o * KVH: (o + n) * KVH])

            pv = ps_pv.tile([P, 32], f32, tag="pv")
            pt = ps_pt.tile([P, KVH], f32, tag="pt")
            z_all = ps_z.tile([1, 32], f32, tag="z")

            for jh in range(KVH // 2):
                ktc = kt_pool.tile([P, 2, n, P], bf16, tag="kt")
                vtc = v_pool.tile([P, 2, n, D], bf16, tag="vt")
                ko = 8 * o * D * P + jh * 2 * D * n * P
                nc.gpsimd.dma_start(
                    ktc,
                    kt_d[0:1, ko: ko + 2 * D * n * P].rearrange(
                        "o (d r) -> (o d) r", d=P))
                vo = 8 * o * P * D + jh * 2 * P * n * D
                nc.gpsimd.dma_start(
                    vtc,
                    vp_d[0:1, vo: vo + 2 * P * n * D].rearrange(
                        "o (p r) -> (o p) r", p=P))

                # QK for the 2 kv heads of this group
                qk = ps_qk.tile([P, 2, n, 4], f32, tag="qk")
                for j2 in range(2):
                    qcol = s * 32 + 4 * (2 * jh + j2)
                    for i in range(n):
                        nc.tensor.matmul(
                            qk[:, j2, i, :],
                            lhsT=ktc[:, j2, i, :],
                            rhs=qt[:, qcol: qcol + 4],
                            start=True, stop=True, skip_group_check=True)

                # softmax pieces, batched over the 2-kvh group
                s1 = work.tile([P, 2, n, 4], f32, tag="s1")
                nc.vector.tensor_mul(
                    s1, qk,
                    ksb_s[:, 2 * jh: 2 * jh + 2].to_broadcast([P, 2, n, 4]))
                e = work.tile([P, 2, n, 4], bf16, tag="e")
                nc.scalar.activation(e, s1, EXP)
                ev = work.tile([P, 2, n, 4], bf16, tag="ev")
                nc.vector.tensor_mul(
                    ev, e,
                    vsb_s[:, 2 * jh: 2 * jh + 2].to_broadcast([P, 2, n, 4]))

                for j2 in range(2):
                    j = 2 * jh + j2
                    # Z partials: per-(tile, head) column sums of e
                    nc.tensor.matmul(
                        pt[0: n * 4, j: j + 1],
                        lhsT=e[:, j2], rhs=ones,
                        start=True, stop=True, skip_group_check=True)
                    # PV accumulate over token tiles: out^T [128d, 4h]
                    cc = 4 * j
                    for i in range(n):
                        nc.tensor.matmul(
                            pv[:, cc: cc + 4],
                            lhsT=vtc[:, j2, i, :],
                            rhs=ev[:, j2, i, :],
                            start=(i == 0), stop=(i == n - 1),
                            skip_group_check=True)

            # fold Z partials -> [1, 32]; ship unnormalized partials to host
            pts = tail.tile([P, KVH], f32, tag="pts")
            nc.vector.tensor_copy(pts[0: n * 4, :], pt[0: n * 4, :])
            for j in range(KVH):
                nc.tensor.matmul(
                    z_all[0:1, 4 * j: 4 * j + 4],
                    lhsT=pts[0: n * 4, j: j + 1],
                    rhs=sel[0: n * 4, :],
                    start=True, stop=True, skip_group_check=True)
            zs = tail.tile([1, 32], f32, tag="zs")
            nc.vector.tensor_copy(zs, z_all)
            nc.scalar.dma_start(z_d[s], zs)
            pvs = tail.tile([P, 32], f32, tag="pvs")
            nc.vector.tensor_copy(pvs, pv)
            nc.scalar.dma_start(pv_d[s], pvs)

    nc.compile()
    return nc


_PROGRAM_CACHE = {}


def _get_program(L):
    key = tuple(L)
    if key not in _PROGRAM_CACHE:
        _PROGRAM_CACHE[key] = _build_program(L)
    return _PROGRAM_CACHE[key]


# ---------------------------------------------------------------------------
# entry point
# ---------------------------------------------------------------------------

def kernel(q, k, v, k_cache_q, v_cache_q, k_scale, v_scale,
           block_tables, context_lens, slot_mapping, _trace=False):
    inputs = dict(q=np.asarray(q), k=np.asarray(k), v=np.asarray(v),
                  k_cache_q=np.asarray(k_cache_q),
                  v_cache_q=np.asarray(v_cache_q),
                  k_scale=np.asarray(k_scale), v_scale=np.asarray(v_scale),
                  block_tables=np.asarray(block_tables),
                  context_lens=np.asarray(context_lens),
                  slot_mapping=np.asarray(slot_mapping))
    L, chunks = _plan(inputs["context_lens"])
    in_maps, padcnt = _pack_inputs(inputs, L, chunks)
    nc = _get_program(L)
    res = run_bass_kernel_spmd(nc, in_maps, core_ids=list(range(NCORES)),
                               trace=_trace)

    # combine unnormalized partials across chunks (flash-decoding merge)
    accp = np.zeros((B, P, 32), dtype=np.float64)
    accz = np.zeros((B, 32), dtype=np.float64)
    for c in range(NCORES):
        pvs = res.results[c]["pv"]   # [SLOTS, P, 32]
        zss = res.results[c]["z"]    # [SLOTS, 1, 32]
        for s in range(len(L)):
            b, _, _ = chunks[s][c]
            accp[b] += pvs[s]
            accz[b] += zss[s][0] - padcnt[c, s]
    out = (accp / accz[:, None, :]).transpose(0, 2, 1)  # [B, 32h, 128d]
    out = np.ascontiguousarray(out.reshape(B, NUM_HEADS * D), dtype=np.float32)
    if _trace:
        return out, res
    return out


# revision 19
# speedup vs baseline: 2.1476x; 1.0092x over previous
"""Trainium2 Bass kernel: paged int8-KV-cache GQA decode attention, 8-core SPMD.

Contract: kernel(**inputs) takes the FULL unsharded numpy inputs (as produced by
the reference setup_inputs) and returns the FULL [32, 4096] float32 output.

Strategy (data parallel over sequence-chunks, flash-decoding style):
  - The 32 sequences' token tiles (ceil(ctx/128) each) are carved into
    8 cores x SLOTS contiguous chunks; slot s has a fixed tile count L[s]
    shared by all cores (SPMD), chosen by a small search to minimize padding
    (sum(L) ~ 6% over the ideal total/8).  A long sequence may span chunks on
    several cores; every chunk computes unnormalized partials (PV^T, Z) and
    the host combines: out = sum(PV) / sum(Z) (softmax without max-shift is
    linear in the partials).
  - Host staging is permutation/layout only: the int8-valued int32 KV cache is
    gathered per block_tables into per-core packed int8 buffers, slot-major so
    each (slot, 4-kvh-group) load is one contiguous DRAM block (128 big DMA
    descriptors).  The new decode token is quantized and scattered exactly as
    the reference's store_kvcache does, before the gather.
  - On device, SWDGE DMAs cast int8 -> bf16 inline during the HBM->SBUF load
    (the DMA engines charge max(src,dst) bytes, so this hits the bf16-landing
    floor of ~45 MB/core); k_scale * softmax_scale and v_scale are folded in
    as per-token vectors after the QK matmul / after exp respectively.
  - Per (slot, group of 4 kv heads):
      scores [128t, 4kvh, n, 4h] = per-tile matmuls(lhsT=K^T tile, rhs=q^T)
      s1 = scores * ksb  (DVE; ksb = k_scale*SCALE, zeroed beyond ctx)
      e  = exp(s1) in bf16 (ACT), one op per 4-kvh group
      ev = e * v_scale_vec (DVE; v_scale zeroed beyond ctx)
      Z  = matmul(lhsT=e, rhs=ones) per kvh -> fold to [1, 32]; pad tokens
           contribute exp(0)=1 each, corrected host-side via the known count
      PV = matmul(lhsT=V tile [128t,128d], rhs=ev [128t,4]) accumulated in
           PSUM as out^T [128d, 4h].
  Softmax skips max-subtraction (scores are O(20) at most; fp32 exp is safe).
"""

import os
import sys
import math
from contextlib import ExitStack

import numpy as np

sys.path.insert(0, "/opt/trn_rl_repo")

import ml_dtypes  # noqa: E402

import concourse.bass as bass  # noqa: E402
import concourse.mybir as mybir  # noqa: E402
import concourse.tile as tile  # noqa: E402
from concourse import bacc  # noqa: E402
from concourse.bass_utils import run_bass_kernel_spmd  # noqa: E402

BF16 = ml_dtypes.bfloat16

B = 32
NUM_HEADS = 32
KVH = 8
D = 128
REP = NUM_HEADS // KVH  # 4
BLOCK_SIZE = 256
T = 4096
P = 128
SCALE = 1.0 / float(np.sqrt(D))
NCORES = 8


# ---------------------------------------------------------------------------
# host-side planning + packing
# ---------------------------------------------------------------------------

def _greedy_chunks(tiles, L):
    """Slot-by-slot, give the 8 largest remaining sequences a chunk of up to
    L[s] tiles.  Returns per-slot lists of (seq, start_tile, len) or None if
    some sequence is left uncovered."""
    rem = [int(t) for t in tiles]
    start = [0] * len(tiles)
    chunks = []
    for Ls in L:
        order = sorted(range(len(tiles)), key=lambda b: -rem[b])
        sc = []
        for c in range(NCORES):
            b = order[c]
            ln = min(rem[b], Ls)
            sc.append((b, start[b], ln))
            rem[b] -= ln
            start[b] += ln
        chunks.append(sc)
    if any(r > 0 for r in rem):
        return None
    return chunks


_PLAN_CACHE = {}


def _plan(context_lens):
    """Choose slot lengths L and the (core, slot) -> sequence-chunk map."""
    tiles = tuple(int(math.ceil(int(c) / P)) for c in context_lens)
    if tiles in _PLAN_CACHE:
        return _PLAN_CACHE[tiles]
    ts = sorted(tiles, reverse=True)
    # octile fallback (always feasible): whole sequences, 4 slots
    best = (ts[0] + ts[8] + ts[16] + ts[24], (ts[0], ts[8], ts[16], ts[24]))
    for L0 in range(max(4, ts[0] - 8), ts[0] + 1):
        for L1 in range(max(4, ts[8] - 8), min(L0, ts[8] + 4) + 1):
            for L2 in range(max(4, ts[16] - 6), min(L1, ts[16] + 4) + 1):
                for L3 in range(max(4, ts[24] - 4), min(L2, ts[24] + 4) + 1):
                    for L4 in range(4, min(L3, 10) + 1):
                        for L5 in (0, *range(4, L4 + 1)):
                            L = (L0, L1, L2, L3, L4) if L5 == 0 else (
                                L0, L1, L2, L3, L4, L5)
                            N = sum(L)
                            if N >= best[0]:
                                continue
                            if _greedy_chunks(tiles, L) is not None:
                                best = (N, L)
    L = list(best[1])
    chunks = _greedy_chunks(tiles, L)
    _PLAN_CACHE[tiles] = (L, chunks)
    return L, chunks


def _quantize(x):
    absmax = np.abs(x).max(axis=-1)
    scale = np.where(absmax > 0.0, absmax / 127.0, 1.0).astype(np.float32)
    xq = np.clip(np.round(x / scale[..., None]), -127.0, 127.0).astype(np.int32)
    return xq, scale


def _pack_inputs(inputs, L, chunks):
    q = inputs["q"].reshape(B, NUM_HEADS, D).astype(np.float32)
    k = inputs["k"].reshape(B, KVH, D).astype(np.float32)
    v = inputs["v"].reshape(B, KVH, D).astype(np.float32)
    kc = np.ascontiguousarray(inputs["k_cache_q"].reshape(-1, KVH, D))
    vc = np.ascontiguousarray(inputs["v_cache_q"].reshape(-1, KVH, D))
    ks = np.ascontiguousarray(inputs["k_scale"].reshape(-1, KVH)).astype(np.float32)
    vs = np.ascontiguousarray(inputs["v_scale"].reshape(-1, KVH)).astype(np.float32)
    bt = inputs["block_tables"]
    ctx = inputs["context_lens"]
    sm = inputs["slot_mapping"]

    # store_kvcache_int8: quantize the new token and scatter into the cache
    kq, ksn = _quantize(k)
    vq, vsn = _quantize(v)
    kc = kc.copy(); vc = vc.copy(); ks = ks.copy(); vs = vs.copy()
    kc[sm] = kq; vc[sm] = vq; ks[sm] = ksn; vs[sm] = vsn

    SLOTS = len(L)
    NTT = sum(L)
    NT = NTT * P
    offs = np.concatenate([[0], np.cumsum(L)])

    in_maps = []
    padcnt = np.zeros((NCORES, SLOTS), dtype=np.float64)
    for c in range(NCORES):
        # K block per (slot, 4-kvh group): [d, j2, tokens]; V: [p, j2, tile, d]
        kt_c = np.zeros((1, KVH * D * NT), dtype=np.int8)
        vp_c = np.zeros((1, KVH * P * NTT * D), dtype=np.int8)
        ksb_c = np.zeros((P, KVH * NTT), dtype=np.float32)
        vsb_c = np.zeros((P, KVH * NTT), dtype=BF16)
        qt_c = np.zeros((P, SLOTS * 32), dtype=np.float32)
        for s in range(SLOTS):
            b, t0, ln = chunks[s][c]
            n = L[s]
            nt = n * P
            o = int(offs[s])
            nvalid = max(0, min(int(ctx[b]) - t0 * P, ln * P))
            padcnt[c, s] = nt - nvalid
            if ln > 0:
                flat = (bt[b][:, None] * BLOCK_SIZE
                        + np.arange(BLOCK_SIZE, dtype=np.int64)[None, :]
                        ).reshape(-1)[t0 * P: t0 * P + ln * P]
                kg = np.zeros((nt, KVH, D), dtype=np.int8)
                vg = np.zeros((nt, KVH, D), dtype=np.int8)
                kg[: ln * P] = kc[flat]
                vg[: ln * P] = vc[flat]
                scg = np.zeros((nt, KVH), dtype=np.float32)
                svg = np.zeros((nt, KVH), dtype=np.float32)
                valid = (np.arange(nt) < nvalid)
                scg[: ln * P] = ks[flat] * SCALE
                svg[: ln * P] = vs[flat]
                scg *= valid[:, None]
                svg *= valid[:, None]
                kjdt = kg.transpose(1, 2, 0)                      # [KVH, D, nt]
                vpjid = vg.reshape(n, P, KVH, D).transpose(1, 2, 0, 3)
                for g in range(KVH // 2):
                    ko = 8 * o * D * P + g * 2 * D * nt
                    kt_c[0, ko: ko + 2 * D * nt] = (
                        kjdt[2 * g: 2 * g + 2].transpose(1, 0, 2).reshape(-1))
                    vo = 8 * o * P * D + g * 2 * P * n * D
                    vp_c[0, vo: vo + 2 * P * n * D] = (
                        vpjid[:, 2 * g: 2 * g + 2].reshape(-1))

                def sprd(a, dt):
                    return a.reshape(n, P, KVH).transpose(1, 2, 0).reshape(
                        P, KVH * n).astype(dt)
                ksb_c[:, o * KVH: (o + n) * KVH] = sprd(scg, np.float32)
                vsb_c[:, o * KVH: (o + n) * KVH] = sprd(svg, BF16)
            qt_c[:, s * 32: (s + 1) * 32] = q[b].transpose(1, 0)  # [D, 32]
        sel = np.tile(np.eye(4, dtype=np.float32), (32, 1))       # [128, 4]
        in_maps.append(dict(kt=kt_c, vp=vp_c, ksb=ksb_c, vsb=vsb_c,
                            qt=qt_c, sel=sel))
    return in_maps, padcnt


# ---------------------------------------------------------------------------
# device program
# ---------------------------------------------------------------------------

def _build_program(L):
    SLOTS = len(L)
    NTT = sum(L)
    NT = NTT * P
    offs = [0]
    for n in L:
        offs.append(offs[-1] + n)
    f32 = mybir.dt.float32
    bf16 = mybir.dt.bfloat16
    i8 = mybir.dt.int8
    EXP = mybir.ActivationFunctionType.Exp

    nc = bacc.Bacc("TRN2", target_bir_lowering=False, debug=False,
                   num_devices=NCORES)

    kt_d = nc.dram_tensor("kt", [1, KVH * D * NT], i8, kind="ExternalInput").ap()
    vp_d = nc.dram_tensor("vp", [1, KVH * P * NTT * D], i8,
                          kind="ExternalInput").ap()
    ksb_d = nc.dram_tensor("ksb", [P, KVH * NTT], f32, kind="ExternalInput").ap()
    vsb_d = nc.dram_tensor("vsb", [P, KVH * NTT], bf16, kind="ExternalInput").ap()
    qt_d = nc.dram_tensor("qt", [P, SLOTS * 32], f32, kind="ExternalInput").ap()
    sel_d = nc.dram_tensor("sel", [P, 4], f32, kind="ExternalInput").ap()
    pv_d = nc.dram_tensor("pv", [SLOTS, P, 32], f32, kind="ExternalOutput").ap()
    z_d = nc.dram_tensor("z", [SLOTS, 1, 32], f32, kind="ExternalOutput").ap()

    with tile.TileContext(nc) as tc, ExitStack() as ctx:
        const = ctx.enter_context(tc.tile_pool(name="const", bufs=1))
        kt_pool = ctx.enter_context(tc.tile_pool(name="ktp", bufs=3))
        v_pool = ctx.enter_context(tc.tile_pool(name="vpp", bufs=3))
        sc_pool = ctx.enter_context(tc.tile_pool(name="scp", bufs=4))
        work = ctx.enter_context(tc.tile_pool(name="wrk", bufs=3))
        tail = ctx.enter_context(tc.tile_pool(name="tl", bufs=2))
        ps_qk = ctx.enter_context(tc.tile_pool(name="psqk", bufs=3, space="PSUM"))
        ps_pt = ctx.enter_context(tc.tile_pool(name="pspt", bufs=2, space="PSUM"))
        ps_z = ctx.enter_context(tc.tile_pool(name="psz", bufs=1, space="PSUM"))
        ps_pv = ctx.enter_context(tc.tile_pool(name="pspv", bufs=2, space="PSUM"))

        qt_f = const.tile([P, SLOTS * 32], f32)
        nc.sync.dma_start(qt_f, qt_d)
        qt = const.tile([P, SLOTS * 32], bf16)
        nc.vector.tensor_copy(qt, qt_f)
        sel = const.tile([P, 4], f32)
        nc.sync.dma_start(sel, sel_d)
        ones = const.tile([P, 1], bf16)
        nc.vector.memset(ones, 1.0)

        # Slots >= RES_START are small; their K/V are loaded ONCE into
        # persistent tiles, enqueued after the pooled slots' loads so the DMA
        # stream never idles waiting on buffer recycling at the tail.
        RES_START = 3 if SLOTS > 4 else SLOTS
        res_tiles = {}

        for s in range(SLOTS):
            n = L[s]
            o = offs[s]
            if s == RES_START:
                for r in range(RES_START, SLOTS):
                    nr = L[r]
                    orr = offs[r]
                    kr = const.tile([P, 4, 2, nr, P], bf16, tag=f"kr{r}")
                    ko = 8 * orr * D * P
                    nc.gpsimd.dma_start(
                        kr, kt_d[0:1, ko: ko + 8 * D * nr * P].rearrange(
                            "o (g d r) -> (o d) g r", g=4, d=P))
                    vr = const.tile([P, 4, 2, nr, D], bf16, tag=f"vr{r}")
                    vo = 8 * orr * P * D
                    nc.gpsimd.dma_start(
                        vr, vp_d[0:1, vo: vo + 8 * P * nr * D].rearrange(
                            "o (g p r) -> (o p) g r", g=4, p=P))
                    res_tiles[r] = (kr, vr)
            ksb_s = sc_pool.tile([P, KVH, n, 1], f32, tag="ksb")
            nc.sync.dma_start(ksb_s, ksb_d[:, o * KVH: (o + n) * KVH])
            vsb_s = sc_pool.tile([P, KVH, n, 1], bf16, tag="vsb")
            nc.sync.dma_start(vsb_s, vsb_d[:, o * KVH: (o + n) * KVH])

            pv = ps_pv.tile([P, 32], f32, tag="pv")
            pt = ps_pt.tile([P, KVH], f32, tag="pt")
            z_all = ps_z.tile([1, 32], f32, tag="z")

            for jh in range(KVH // 2):
                if s >= RES_START:
                    ktc = res_tiles[s][0][:, jh]
                    vtc = res_tiles[s][1][:, jh]
                else:
                    ktc = kt_pool.tile([P, 2, n, P], bf16, tag="kt")
                    vtc = v_pool.tile([P, 2, n, D], bf16, tag="vt")
                    ko = 8 * o * D * P + jh * 2 * D * n * P
                    nc.gpsimd.dma_start(
                        ktc,
                        kt_d[0:1, ko: ko + 2 * D * n * P].rearrange(
                            "o (d r) -> (o d) r", d=P))
                    vo = 8 * o * P * D + jh * 2 * P * n * D
                    nc.gpsimd.dma_start(
                        vtc,
                        vp_d[0:1, vo: vo + 2 * P * n * D].rearrange(
                            "o (p r) -> (o p) r", p=P))

                # QK for the 2 kv heads of this group
                qk = ps_qk.tile([P, 2, n, 4], f32, tag="qk")
                for j2 in range(2):
                    qcol = s * 32 + 4 * (2 * jh + j2)
                    for i in range(n):
                        nc.tensor.matmul(
                            qk[:, j2, i, :],
                            lhsT=ktc[:, j2, i, :],
                            rhs=qt[:, qcol: qcol + 4],
                            start=True, stop=True, skip_group_check=True)

                # softmax pieces, batched over the 2-kvh group
                s1 = work.tile([P, 2, n, 4], f32, tag="s1")
                nc.vector.tensor_mul(
                    s1, qk,
                    ksb_s[:, 2 * jh: 2 * jh + 2].to_broadcast([P, 2, n, 4]))
                e = work.tile([P, 2, n, 4], bf16, tag="e")
                nc.scalar.activation(e, s1, EXP)
                ev = work.tile([P, 2, n, 4], bf16, tag="ev")
                nc.vector.tensor_mul(
                    ev, e,
                    vsb_s[:, 2 * jh: 2 * jh + 2].to_broadcast([P, 2, n, 4]))

                for j2 in range(2):
                    j = 2 * jh + j2
                    # Z partials: per-(tile, head) column sums of e
                    nc.tensor.matmul(
                        pt[0: n * 4, j: j + 1],
                        lhsT=e[:, j2], rhs=ones,
                        start=True, stop=True, skip_group_check=True)
                    # PV accumulate over token tiles: out^T [128d, 4h]
                    cc = 4 * j
                    for i in range(n):
                        nc.tensor.matmul(
                            pv[:, cc: cc + 4],
                            lhsT=vtc[:, j2, i, :],
                            rhs=ev[:, j2, i, :],
                            start=(i == 0), stop=(i == n - 1),
                            skip_group_check=True)

            # fold Z partials -> [1, 32]; ship unnormalized partials to host
            pts = tail.tile([P, KVH], f32, tag="pts")
            nc.vector.tensor_copy(pts[0: n * 4, :], pt[0: n * 4, :])
            for j in range(KVH):
                nc.tensor.matmul(
                    z_all[0:1, 4 * j: 4 * j + 4],
                    lhsT=pts[0: n * 4, j: j + 1],
                    rhs=sel[0: n * 4, :],
                    start=True, stop=True, skip_group_check=True)
            zs = tail.tile([1, 32], f32, tag="zs")
            nc.vector.tensor_copy(zs, z_all)
            nc.scalar.dma_start(z_d[s], zs)
            pvs = tail.tile([P, 32], f32, tag="pvs")
            nc.vector.tensor_copy(pvs, pv)
            nc.scalar.dma_start(pv_d[s], pvs)

    nc.compile()
    return nc


_PROGRAM_CACHE = {}


def _get_program(L):
    key = tuple(L)
    if key not in _PROGRAM_CACHE:
        _PROGRAM_CACHE[key] = _build_program(L)
    return _PROGRAM_CACHE[key]


# ---------------------------------------------------------------------------
# entry point
# ---------------------------------------------------------------------------

def kernel(q, k, v, k_cache_q, v_cache_q, k_scale, v_scale,
           block_tables, context_lens, slot_mapping, _trace=False):
    inputs = dict(q=np.asarray(q), k=np.asarray(k), v=np.asarray(v),
                  k_cache_q=np.asarray(k_cache_q),
                  v_cache_q=np.asarray(v_cache_q),
                  k_scale=np.asarray(k_scale), v_scale=np.asarray(v_scale),
                  block_tables=np.asarray(block_tables),
                  context_lens=np.asarray(context_lens),
                  slot_mapping=np.asarray(slot_mapping))
    L, chunks = _plan(inputs["context_lens"])
    in_maps, padcnt = _pack_inputs(inputs, L, chunks)
    nc = _get_program(L)
    res = run_bass_kernel_spmd(nc, in_maps, core_ids=list(range(NCORES)),
                               trace=_trace)

    # combine unnormalized partials across chunks (flash-decoding merge)
    accp = np.zeros((B, P, 32), dtype=np.float64)
    accz = np.zeros((B, 32), dtype=np.float64)
    for c in range(NCORES):
        pvs = res.results[c]["pv"]   # [SLOTS, P, 32]
        zss = res.results[c]["z"]    # [SLOTS, 1, 32]
        for s in range(len(L)):
            b, _, _ = chunks[s][c]
            accp[b] += pvs[s]
            accz[b] += zss[s][0] - padcnt[c, s]
    out = (accp / accz[:, None, :]).transpose(0, 2, 1)  # [B, 32h, 128d]
    out = np.ascontiguousarray(out.reshape(B, NUM_HEADS * D), dtype=np.float32)
    if _trace:
        return out, res
    return out
